# revision 36
# baseline (speedup 1.0000x reference)
import os
import sys

import numpy as np

sys.path.insert(0, "/opt/trn_rl_repo")

import concourse.bacc as bacc
import concourse.bass as bass
import concourse.mybir as mybir
import concourse.tile as tile
from concourse.bass_utils import run_bass_kernel_spmd

# ----- problem config (hardcoded from spec) -----
B = 2048
NUM_GENES = 4096
N_CORES = 8
BS = B // N_CORES  # 256 batch per core
EPS = 1e-5
# (stratum, n_terms, input_dim, output_dim, genes_per_term)
CFG = [(4, 256, 16, 20, 16), (3, 64, 144, 20, 64), (2, 16, 336, 77, 256),
       (1, 4, 1332, 308, 1024), (0, 1, 5328, 1229, 4096)]

F16 = mybir.dt.float16
F32 = mybir.dt.float32

# padded per-term output slots (includes head columns inside the slot)
SLOT = {4: 32, 3: 32, 2: 128, 1: 320, 0: 1280}
NTILES = {4: 64, 3: 16, 2: 16, 1: 10, 0: 10}  # channel tiles of 128 rows per stratum

LAST_RESULTS = None


# ============================================================
# Host-side packing of weights into PE lhsT tile layouts (fp16)
# ============================================================
def _pack(inp):
    W4, W3, W2, W1, W0 = inp["W4"], inp["W3"], inp["W2"], inp["W1"], inp["W0"]
    hw4, hw3, hw2, hw1, hw0 = inp["hw4"], inp["hw3"], inp["hw2"], inp["hw1"], inp["hw0"]
    p = {}

    # s4: 64 groups of 4 terms, paired: [32, 128, 256]
    w4p = np.zeros((32, 128, 256), np.float32)
    for k in range(32):
        for a in range(2):           # group 2k+a
            g = 2 * k + a
            for t in range(4):       # term within group
                term = 4 * g + t
                w4p[k, 64 * a + 16 * t:64 * a + 16 * t + 16,
                    128 * a + 32 * t:128 * a + 32 * t + 20] = W4[term]
    p["w4p"] = w4p

    # s3 gene: strip per 2-term group, full-M 128 cols of the h3 tile
    w3g = np.zeros((32, 128, 128), np.float32)
    for G in range(32):
        for a in range(2):
            term = 2 * G + a
            w3g[G, 64 * a:64 * a + 64, 32 * (term % 4):32 * (term % 4) + 20] = \
                W3[term, 80:144, :]
    p["w3g"] = w3g

    # s3 act: per term, full-M 128 cols; rows = act4T children tile (4 x 32)
    w3a = np.zeros((64, 128, 128), np.float32)
    for j in range(64):
        base = 32 * (j % 4)
        for c in range(4):
            w3a[j, 32 * c:32 * c + 20, base:base + 20] = W3[j, 20 * c:20 * c + 20, :]
            w3a[j, 32 * c:32 * c + 20, base + 20 + c] = hw4[4 * j + c, :, 0]
    p["w3a"] = w3a

    # s2 gene: [32, 128, 128] (term j chunk c2 at idx 2j+c2)
    w2g = np.zeros((32, 128, 128), np.float32)
    for j in range(16):
        for c2 in range(2):
            w2g[2 * j + c2, :, 0:77] = W2[j, 80 + 128 * c2:80 + 128 * c2 + 128, :]
    p["w2g"] = w2g

    # s2 act: [16, 128, 128]; rows = act3T tile (4 children x 32)
    w2a = np.zeros((16, 128, 128), np.float32)
    for j in range(16):
        for c in range(4):
            w2a[j, 32 * c:32 * c + 20, 0:77] = W2[j, 20 * c:20 * c + 20, :]
            w2a[j, 32 * c:32 * c + 20, 77 + c] = hw3[4 * j + c, :, 0]  # s3 head
    p["w2a"] = w2a

    # s1: term j covers h1 rows 320j..320j+319 across 3 tiles; strips hold
    # 3 full-M 128-col blocks (tile-aligned, zero-padded)
    S1_BASE_T = [0, 2, 5, 7]

    def s1_cols(j, o):
        R = 320 * j + o
        return 128 * (R // 128 - S1_BASE_T[j]) + R % 128

    cols308 = {j: np.array([s1_cols(j, o) for o in range(308)]) for j in range(4)}
    w1g = np.zeros((32, 128, 384), np.float32)
    for j in range(4):
        for c in range(8):
            w1g[8 * j + c][:, cols308[j]] = W1[j, 308 + 128 * c:308 + 128 * c + 128, :]
    p["w1g"] = w1g

    w1a = np.zeros((16, 128, 384), np.float32)
    for j in range(4):
        for c in range(4):
            w1a[4 * j + c][0:77, cols308[j]] = W1[j, 77 * c:77 * c + 77, :]
            w1a[4 * j + c][0:77, s1_cols(j, 308 + c)] = hw2[4 * j + c, :, 0]
    p["w1a"] = w1a

    # s0 gene: split into two 5-out-tile sweep tensors (each streamed once)
    w0g = np.zeros((32, 128, 1280), np.float32)
    for c in range(32):
        w0g[c, :, 0:1229] = W0[0, 1232 + 128 * c:1232 + 128 * c + 128, :]
    p["w0ga"] = np.ascontiguousarray(w0g[:, :, :640])
    p["w0gb"] = np.ascontiguousarray(w0g[:, :, 640:])

    # s0 act: [10, 128, 1280]; rows = act1T (4 terms x 320, 308 real)
    w0a = np.zeros((10, 128, 1280), np.float32)
    for c in range(10):
        for r in range(128):
            R = 128 * c + r
            j, rr = R // 320, R % 320
            if rr < 308:
                w0a[c, r, 0:1229] = W0[0, 308 * j + rr, :]
                w0a[c, r, 1229 + j] = hw1[j, rr, 0]  # s1 head
    p["w0aa"] = np.ascontiguousarray(w0a[:, :, :640])
    p["w0ab"] = np.ascontiguousarray(w0a[:, :, 640:])

    # head-extraction selection matrices
    sel3 = np.zeros((16, 128, 128), np.float32)
    for j2 in range(16):
        for jj in range(4):
            for c in range(4):
                sel3[j2, 32 * jj + 20 + c, 16 * (j2 % 8) + 4 * jj + c] = 1.0
    p["sel3"] = sel3
    sel2 = np.zeros((16, 128, 128), np.float32)
    for j in range(16):
        for c in range(4):
            sel2[j, 77 + c, 4 * j + c] = 1.0
    p["sel2"] = sel2
    sel1 = np.zeros((4, 128, 128), np.float32)
    rowk = [52, 116, 52, 116]
    for k in range(4):
        for c in range(4):
            sel1[k, rowk[k] + c, 64 + 4 * k + c] = 1.0
    p["sel1"] = sel1
    sel0 = np.zeros((1, 128, 128), np.float32)
    for c in range(4):
        sel0[0, 77 + c, 80 + c] = 1.0
    p["sel0"] = sel0

    hw0p = np.zeros((10, 128, 1), np.float32)
    for c in range(10):
        n = min(128, 1229 - 128 * c)
        if n > 0:
            hw0p[c, :n, 0] = hw0[0, 128 * c:128 * c + n, 0]
    p["hw0p"] = hw0p

    p["eye"] = np.eye(128, dtype=np.float32)[None]

    out16 = {}
    for k, v in p.items():
        v16 = v.astype(np.float16)
        if k in _ILV_N:
            v16 = _ilv(v16, _ILV_N[k])
        out16[k] = np.ascontiguousarray(v16)

    # g / bb channel-tiled f32 vectors [128, ntiles] (pad rows -> 0)
    def tile_vec(vec_f, s):
        nt = NTILES[s]
        slot = SLOT[s]
        out = np.zeros((128, nt), np.float32)
        for T in range(nt):
            for prt in range(128):
                R = 128 * T + prt
                j, o = R // slot, R % slot
                v = vec_f(j, o)
                if v is not None:
                    out[prt, T] = v
        return out

    gts, bbs = [], []
    for s, T_, I_, O_, _ in CFG:
        g, bb = inp[f"g{s}"], inp[f"bb{s}"]
        gts.append(tile_vec(lambda j, o: g[j, o] if (j < T_ and o < O_) else None, s))
        bbs.append(tile_vec(lambda j, o: bb[j, o] if (j < T_ and o < O_) else None, s))
    out16["gtall"] = np.ascontiguousarray(np.concatenate(gts, axis=1))
    out16["bball"] = np.ascontiguousarray(np.concatenate(bbs, axis=1))
    return out16


def _ilv(arr, n):
    """[nb, 128, F] -> [nb//n, 128, n*F]: n strips side-by-side per partition
    (matches the SBUF tile layout, so DMAs need no rearrange and get one
    contiguous chunk per partition)."""
    nb, p, f = arr.shape
    assert nb % n == 0
    return np.ascontiguousarray(
        arr.reshape(nb // n, n, p, f).transpose(0, 2, 1, 3).reshape(nb // n, p, n * f))

_ILV_N = {"w4p": 4, "w3g": 4, "w3a": 8, "w2g": 4, "w2a": 4, "w1g": 8,
          "w1a": 4, "w0ga": 2, "w0gb": 2, "w0aa": 2, "w0ab": 2, "sel3": 4, "sel2": 4,
          "sel1": 4, "hw0p": 10}


# ============================================================
# Bass program (built once, shared across calls)
# ============================================================
_NC = None

# s1 out-piece map: term j -> list of (tile, row_base, width) covering rows 320j..320j+319
S1_PIECES = {
    0: [(0, 0, 128), (1, 0, 128), (2, 0, 64)],
    1: [(2, 64, 64), (3, 0, 128), (4, 0, 128)],
    2: [(5, 0, 128), (6, 0, 128), (7, 0, 64)],
    3: [(7, 64, 64), (8, 0, 128), (9, 0, 128)],
}
TILE_LAST_TERM = {0: 0, 1: 0, 2: 1, 3: 1, 4: 1, 5: 2, 6: 2, 7: 3, 8: 3, 9: 3}
TILE_FIRST_TERM = {0: 0, 1: 0, 2: 0, 3: 1, 4: 1, 5: 2, 6: 2, 7: 2, 8: 3, 9: 3}
S1_BASE_T = [0, 2, 5, 7]


def _build():
    nc = bacc.Bacc("TRN2", target_bir_lowering=False, debug=False,
                   enable_asserts=True, num_devices=N_CORES)
    dep = bass._add_dep_helper
    io = {}
    io["xt"] = nc.dram_tensor("xt", [4, 128, 8 * BS], F16, kind="ExternalInput")
    for name, shp in [("w4p", [8, 128, 1024]), ("w3g", [8, 128, 512]),
                      ("w3a", [8, 128, 1024]), ("w2g", [8, 128, 512]),
                      ("w2a", [4, 128, 512]), ("w1g", [4, 128, 3072]),
                      ("w1a", [4, 128, 1536]), ("w0ga", [16, 128, 1280]),
                      ("w0gb", [16, 128, 1280]), ("w0aa", [5, 128, 1280]),
                      ("w0ab", [5, 128, 1280]),
                      ("sel3", [4, 128, 512]), ("sel2", [4, 128, 512]),
                      ("sel1", [1, 128, 512]), ("sel0", [1, 128, 128]),
                      ("hw0p", [1, 128, 10]), ("eye", [1, 128, 128])]:
        io[name] = nc.dram_tensor(name, shp, F16, kind="ExternalInput")
    NTOT = sum(NTILES.values())
    io["gtall"] = nc.dram_tensor("gtall", [128, NTOT], F32, kind="ExternalInput")
    io["bball"] = nc.dram_tensor("bball", [128, NTOT], F32, kind="ExternalInput")
    y = nc.dram_tensor("y", [4, 128, BS], F32, kind="ExternalOutput")

    rg = [list(range(N_CORES))]

    with tile.TileContext(nc, num_cores=N_CORES) as tc:
        with tc.tile_pool(name="per", bufs=1) as per, \
             tc.tile_pool(name="wp", bufs=3) as wp, \
             tc.tile_pool(name="pp", bufs=7, space="PSUM") as pp, \
             tc.tile_pool(name="dp", bufs=1, space="DRAM") as dp:

            # ---- persistent SBUF ----
            xsb = per.tile([128, 32 * BS], F16, name="xsb", tag="xsb")
            actT = {s: per.tile([128, NTILES[s] * BS], F16, name=f"act{s}", tag=f"act{s}")
                    for s in (4, 3, 2, 1, 0)}
            hT = {s: per.tile([128, NTILES[s] * BS], F16, name=f"h{s}", tag=f"h{s}")
                  for s in (3, 2, 1, 0)}
            h1gene = per.tile([128, 10 * BS], F16, name="h1gene", tag="h1gene")
            h0gene = per.tile([128, 10 * BS], F16, name="h0gene", tag="h0gene")
            stats = {s: per.tile([128, NTILES[s] * 6], F32, name=f"st{s}", tag=f"st{s}")
                     for s in (4, 3, 2, 1, 0)}
            agg = per.tile([128, 8 * NTILES[4] * 2], F32, name="agg", tag="agg")
            ccs = per.tile([128, NTILES[4] * 2], F32, name="ccs", tag="ccs")
            prtmp = per.tile([128, NTILES[4] * 2], F32, name="prtmp", tag="prtmp")
            prtmp2 = per.tile([128, NTILES[4]], F32, name="prtmp2", tag="prtmp2")
            aT = {s: per.tile([128, NTILES[s]], F32, name=f"aT{s}", tag=f"aT{s}")
                  for s in (4, 3, 2, 1, 0)}
            cT = {s: per.tile([128, NTILES[s]], F32, name=f"cT{s}", tag=f"cT{s}")
                  for s in (4, 3, 2, 1, 0)}
            sd_t = {s: per.tile([128, NTILES[s]], F32, name=f"sd{s}", tag=f"sd{s}")
                    for s in (4, 3, 2, 1, 0)}
            mub = per.tile([128, NTILES[4]], F32, name="mub", tag="mub")
            varb = per.tile([128, NTILES[4]], F32, name="varb", tag="varb")
            gsb_all = per.tile([128, 116], F32, name="gsb_all", tag="gsb_all")
            bbsb_all = per.tile([128, 116], F32, name="bbsb_all", tag="bbsb_all")
            _off = {}
            _o = 0
            for s in (4, 3, 2, 1, 0):
                _off[s] = _o
                _o += NTILES[s]
            gsb = {s: gsb_all[:, _off[s]:_off[s] + NTILES[s]] for s in (4, 3, 2, 1, 0)}
            bbsb = {s: bbsb_all[:, _off[s]:_off[s] + NTILES[s]] for s in (4, 3, 2, 1, 0)}
            outsb = per.tile([128, 4, BS], F32, name="outsb", tag="outsb")
            zbuf = per.tile([128, 8 * BS], F16, name="zbuf", tag="zbuf")
            eps_sb = per.tile([128, 1], F32, name="eps_sb", tag="eps_sb")
            eyesb = per.tile([128, 128], F16, name="eyesb", tag="eyesb")
            nc.vector.memset(eps_sb[:], EPS)
            nc.vector.memset(outsb[:, 3, :], 0.0)

            # ---- front DMAs: xt spread over 3 queues (ACT kept for w4t) ----
            nc.sync.dma_start(xsb[:, 0:8 * BS], io["xt"][0])
            nc.gpsimd.dma_start(xsb[:, 8 * BS:16 * BS], io["xt"][1])
            nc.gpsimd.dma_start(xsb[:, 16 * BS:24 * BS], io["xt"][2])
            nc.gpsimd.dma_start(xsb[:, 24 * BS:32 * BS], io["xt"][3])
            nc.sync.dma_start(gsb_all[:], io["gtall"][:])
            nc.sync.dma_start(bbsb_all[:], io["bball"][:])
            nc.sync.dma_start(eyesb[:], io["eye"][0])

            def xtile(t):
                return xsb[:, BS * t:BS * (t + 1)]

            def acttile(s, t):
                return actT[s][:, BS * t:BS * (t + 1)]

            def htile(s, t):
                return hT[s][:, BS * t:BS * (t + 1)]

            def chain(mms):
                for a, b in zip(mms, mms[1:]):
                    dep(b.ins, a.ins, sync=False,
                        reason="psum accumulation order")

            # slim stats prereduce + collective + postmath
            # stats layout: one 6-field record per tile, halves of 128:
            # (c0, m0, M2_0, c1, m1, M2_1)
            def prereduce(s, lo=0, n=None, coff=0):
                n = NTILES[s] if n is None else n
                sv = stats[s][:, 6 * lo:6 * (lo + n)].rearrange(
                    "p (t e th) -> p t e th", e=2, th=3)
                mv = sv[:, :, :, 1]   # per-half means (128 samples) [128, n, 2]
                vv = sv[:, :, :, 2]   # per-half M2 sums             [128, n, 2]
                msq = prtmp[:, :2 * n].rearrange("p (t e) -> p t e", e=2)
                f0 = ccs[:, coff:coff + n]
                f1 = ccs[:, coff + n:coff + 2 * n]
                nc.vector.tensor_tensor(msq, mv, mv, op=mybir.AluOpType.mult)
                nc.vector.tensor_tensor(f0, mv[:, :, 0], mv[:, :, 1],
                                        op=mybir.AluOpType.add)
                nc.vector.tensor_tensor(prtmp2[:, :n], msq[:, :, 0], msq[:, :, 1],
                                        op=mybir.AluOpType.add)
                nc.vector.tensor_scalar_mul(prtmp2[:, :n], prtmp2[:, :n], 128.0)
                nc.vector.tensor_tensor(f1, vv[:, :, 0], vv[:, :, 1],
                                        op=mybir.AluOpType.add)
                nc.vector.tensor_tensor(f1, f1, prtmp2[:, :n],
                                        op=mybir.AluOpType.add)

            def bn_collective(s, n=None, coff=0, aoff=0, suf=""):
                n = NTILES[s] if n is None else n
                F = 2 * n
                cc_in = dp.tile([128, F], F32, name=f"cci{s}{suf}")
                cc_out = dp.tile([N_CORES, 128, F], F32, name=f"cco{s}{suf}",
                                 addr_space="Shared")
                nc.gpsimd.dma_start(cc_in[:], ccs[:, coff:coff + F])
                nc.gpsimd.collective_compute(
                    "AllGather", mybir.AluOpType.bypass, replica_groups=rg,
                    ins=[cc_in.opt()], outs=[cc_out.opt()])
                nc.gpsimd.dma_start(agg[:, aoff:aoff + 8 * F],
                                    cc_out.rearrange("c p f -> p c f"))

            def postmath(s, lo=0, n=None, aoff=0):
                n = NTILES[s] if n is None else n
                F = 2 * n
                ag = agg[:, aoff:aoff + 8 * F]
                nc.vector.tensor_tensor(ag[:, 0:4 * F], ag[:, 0:4 * F],
                                        ag[:, 4 * F:8 * F], op=mybir.AluOpType.add)
                nc.vector.tensor_tensor(ag[:, 0:2 * F], ag[:, 0:2 * F],
                                        ag[:, 2 * F:4 * F], op=mybir.AluOpType.add)
                nc.vector.tensor_tensor(ag[:, 0:F], ag[:, 0:F],
                                        ag[:, F:2 * F], op=mybir.AluOpType.add)
                mu = mub[:, lo:lo + n]
                va = varb[:, lo:lo + n]
                sd = sd_t[s][:, lo:lo + n]
                nc.vector.tensor_scalar_mul(mu, ag[:, 0:n], 1.0 / 16)
                nc.vector.tensor_scalar_mul(va, ag[:, n:F], 1.0 / 2048)
                nc.vector.tensor_tensor(prtmp2[:, :n], mu, mu,
                                        op=mybir.AluOpType.mult)
                nc.vector.tensor_tensor(va, va, prtmp2[:, :n],
                                        op=mybir.AluOpType.subtract)
                nc.scalar.activation(sd, va,
                                     mybir.ActivationFunctionType.Sqrt,
                                     bias=eps_sb[:, 0:1])
                nc.vector.reciprocal(sd, sd)
                nc.vector.tensor_tensor(aT[s][:, lo:lo + n], sd,
                                        gsb[s][:, lo:lo + n],
                                        op=mybir.AluOpType.mult)
                nc.vector.tensor_tensor(sd, mu, aT[s][:, lo:lo + n],
                                        op=mybir.AluOpType.mult)
                nc.vector.tensor_tensor(cT[s][:, lo:lo + n],
                                        bbsb[s][:, lo:lo + n], sd,
                                        op=mybir.AluOpType.subtract)

            def tanh_tile(s, t, src):
                nc.scalar.activation(
                    acttile(s, t), src,
                    mybir.ActivationFunctionType.Tanh,
                    bias=cT[s][:, t:t + 1], scale=aT[s][:, t:t + 1])

            def copy_stat(s, t, ps):
                # h tile: PSUM f32 -> SBUF fp16 (ACT/DVE alternate; gpsimd
                # cannot read PSUM), per-tile fp16 bn_stats on DVE
                if t % 2 == 0:
                    nc.scalar.copy(htile(s, t), ps[:])
                else:
                    nc.vector.tensor_copy(htile(s, t), ps[:])
                nc.vector.bn_stats(stats[s][:, 6 * t:6 * t + 6], htile(s, t))

            # ================= s4 pass 1: stats only =================
            def s4_mm(t, ps, wt):
                a = t % 2
                rhs = xsb[64 * a:64 * a + 64, BS * (t // 2):BS * (t // 2) + BS]
                return nc.tensor.matmul(
                    ps[:], wt[64 * a:64 * a + 64, 128 * a:128 * a + 128],
                    rhs, start=True, stop=True)

            w4keep = {}

            def s4p1_range(P0, P1):
                for P in range(P0, P1):
                    if P % 4 == 0:
                        w4t = wp.tile([128, 1024], F16, name="w4t", tag="w4",
                                      bufs=8)
                        nc.sync.dma_start(w4t[:], io["w4p"][P // 4])
                        w4keep[P // 4] = w4t
                    w4v = w4t[:, 256 * (P % 4):256 * (P % 4) + 256]
                    for a in range(2):
                        t = 2 * P + a
                        ps = pp.tile([128, BS], F32, name="ps4", tag="ps")
                        s4_mm(t, ps, w4v)
                        if t % 4 == 0:
                            # direct f32 stats from PSUM on DVE (no copy)
                            nc.vector.bn_stats(stats[4][:, 6 * t:6 * t + 6],
                                               ps[:])
                        else:
                            sc = acttile(4, t)
                            nc.scalar.copy(sc, ps[:])
                            nc.vector.bn_stats(stats[4][:, 6 * t:6 * t + 6], sc)

            # s4 stats in two halves; half-A collective fires at mid-front so
            # half-A tanh + s3-A overlap the half-B collective
            s4p1_range(0, 16)
            prereduce(4, lo=0, n=32, coff=0)
            bn_collective(4, n=32, coff=0, aoff=0, suf="a")
            s4p1_range(16, 32)
            prereduce(4, lo=32, n=32, coff=64)
            bn_collective(4, n=32, coff=64, aoff=512, suf="b")

            # ================= s1 gene (under coll4) =================
            ps1 = {}
            mms1 = {}
            for j in range(4):
                for (tl, rb, w) in S1_PIECES[j]:
                    if tl not in ps1:
                        ps1[tl] = pp.tile([128, BS], F32, name=f"ps1g{tl}", tag="ps")
                        mms1[tl] = []
                w1t = wp.tile([128, 8 * 384], F16, name="w1t", tag="w1", bufs=2)
                nc.sync.dma_start(w1t[:], io["w1g"][j])
                for c in range(8):
                    for (tl, rb, w) in S1_PIECES[j]:
                        lt = tl - S1_BASE_T[j]
                        mms1[tl].append(nc.tensor.matmul(
                            ps1[tl][:], w1t[:, 384 * c + 128 * lt:384 * c + 128 * lt + 128],
                            xtile(8 * j + c),
                            start=(c == 0 and j == TILE_FIRST_TERM[tl]),
                            stop=(c == 7 and j == TILE_LAST_TERM[tl])))
                for tl, lt in TILE_LAST_TERM.items():
                    if lt == j and tl in ps1:
                        chain(mms1[tl])
                        if tl % 2 == 0:
                            nc.scalar.copy(h1gene[:, BS * tl:BS * (tl + 1)],
                                           ps1[tl][:])
                        else:
                            nc.vector.tensor_copy(
                                h1gene[:, BS * tl:BS * (tl + 1)], ps1[tl][:])
                        del ps1[tl]


            # ================= gap4: s4 pass 2 + s3 interleaved =================
            # (w4p pass 2 / w3g / w3a stream on the SP queue, idle in this phase)
            def gap4_range(P0, P1):
              for P in range(P0, P1):
                w4t2 = w4keep[P // 4]
                w4v2 = w4t2[:, 256 * (P % 4):256 * (P % 4) + 256]
                for a in range(2):
                    t = 2 * P + a
                    ps = pp.tile([128, BS], F32, name="ps4b", tag="ps")
                    s4_mm(t, ps, w4v2)
                    if P % 2 == 0:
                        # first half of each 4-tile group: DVE prescale into
                        # zbuf (fast PSUM release), wide plain tanh later
                        zs = (t // 4 % 2) * 2 + t % 4
                        zslot = zbuf[:, BS * zs:BS * (zs + 1)]
                        nc.vector.tensor_scalar(zslot, ps[:], aT[4][:, t:t + 1],
                                                cT[4][:, t:t + 1],
                                                op0=mybir.AluOpType.mult,
                                                op1=mybir.AluOpType.add)
                    else:
                        # second half: fused scale/bias tanh from PSUM on ACT
                        tanh_tile(4, t, ps[:])
                if P % 2 == 1:
                    t3 = P // 2
                    # plain wide tanh over the 2 prescaled tiles 4*t3, 4*t3+1
                    h = ((t3 % 2) * 2) * BS
                    nc.scalar.activation(
                        actT[4][:, BS * 4 * t3:BS * (4 * t3 + 2)],
                        zbuf[:, h:h + 2 * BS],
                        mybir.ActivationFunctionType.Tanh)
                    if t3 % 2 == 0:
                        w3at = wp.tile([128, 1024], F16, name="w3at", tag="w3a", bufs=4)
                        nc.sync.dma_start(w3at[:], io["w3a"][t3 // 2])
                        w3gt = wp.tile([128, 512], F16, name="w3gt", tag="w3g", bufs=4)
                        nc.sync.dma_start(w3gt[:], io["w3g"][t3 // 2])
                    ps = pp.tile([128, BS], F32, name="ps3", tag="ps")
                    mms = []
                    for a in range(2):  # gene groups (full-M padded)
                        G = 2 * t3 + a
                        goff = 128 * (G % 4)
                        mms.append(nc.tensor.matmul(
                            ps[:], w3gt[:, goff:goff + 128], xtile(G),
                            start=(a == 0), stop=False))
                    for jj in range(4):  # act terms (full-M padded)
                        j = 4 * t3 + jj
                        k = 128 * (j % 8)
                        mms.append(nc.tensor.matmul(
                            ps[:], w3at[:, k:k + 128],
                            acttile(4, j), start=False, stop=(jj == 3)))
                    chain(mms)
                    copy_stat(3, t3, ps)
            postmath(4, lo=0, n=32, aoff=0)
            gap4_range(0, 16)
            postmath(4, lo=32, n=32, aoff=512)
            gap4_range(16, 32)

            prereduce(3)
            bn_collective(3)

            # ================= under coll3: s4 heads + s0g sweep A =================
            w2ah = wp.tile([128, 4 * 512], F16, name="w2ah", tag="w2a", bufs=1)
            st3h = wp.tile([128, 4 * 512], F16, name="st3h", tag="sel3", bufs=1)
            for n in range(4):
                nc.scalar.dma_start(w2ah[:, 512 * n:512 * (n + 1)], io["w2a"][n])
                nc.scalar.dma_start(st3h[:, 512 * n:512 * (n + 1)], io["sel3"][n])

            psA = pp.tile([128, BS], F32, name="psA", tag="ps")
            psB = pp.tile([128, BS], F32, name="psB", tag="ps")
            mmsA, mmsB = [], []
            for j2 in range(16):
                mm = nc.tensor.matmul(
                    (psA if j2 < 8 else psB)[:],
                    st3h[:, 512 * (j2 // 4) + 128 * (j2 % 4):
                         512 * (j2 // 4) + 128 * (j2 % 4) + 128],
                    htile(3, j2),
                    start=(j2 % 8 == 0), stop=(j2 % 8 == 7))
                (mmsA if j2 < 8 else mmsB).append(mm)
            chain(mmsA)
            chain(mmsB)
            nc.scalar.copy(outsb[:, 0, :], psA[:])
            nc.scalar.copy(outsb[:, 1, :], psB[:])
            nc.scalar.dma_start(y[0], outsb[:, 0, :])
            nc.scalar.dma_start(y[1], outsb[:, 1, :])

            # s0g sweep A: fills the coll3 window
            psga = [pp.tile([128, BS], F32, name=f"ps0ga{m}", tag="ps")
                    for m in range(5)]
            mmsga = [[] for _ in range(5)]
            for c in range(32):
                if c % 2 == 0:
                    w0t = wp.tile([128, 1280], F16, name="w0t", tag="w0", bufs=3)
                    nc.sync.dma_start(w0t[:], io["w0ga"][c // 2])
                base = 640 * (c % 2)
                for m in range(5):
                    mmsga[m].append(nc.tensor.matmul(
                        psga[m][:], w0t[:, base + 128 * m:base + 128 * m + 128],
                        xtile(c), start=(c == 0), stop=(c == 31)))
            for m in range(5):
                chain(mmsga[m])
                nc.scalar.copy(h0gene[:, BS * m:BS * (m + 1)], psga[m][:])

            postmath(3)

            # ================= gap3: act3 + s2 =================
            for j in range(16):
                if j % 2 == 0:
                    w2gt = wp.tile([128, 512], F16, name="w2gt", tag="w2g", bufs=2)
                    nc.gpsimd.dma_start(w2gt[:], io["w2g"][j // 2])
                tanh_tile(3, j, htile(3, j))
                ps = pp.tile([128, BS], F32, name="ps2", tag="ps")
                mms = []
                for c2 in range(2):
                    goff = 128 * ((2 * j + c2) % 4)
                    mms.append(nc.tensor.matmul(
                        ps[:], w2gt[:, goff:goff + 128], xtile(2 * j + c2),
                        start=(c2 == 0), stop=False))
                aoff = 512 * (j // 4) + 128 * (j % 4)
                mms.append(nc.tensor.matmul(
                    ps[:], w2ah[:, aoff:aoff + 128], acttile(3, j),
                    start=False, stop=True))
                chain(mms)
                copy_stat(2, j, ps)
            # s0g sweep B: fills the cci2/coll2 window
            psgb = [pp.tile([128, BS], F32, name=f"ps0gb{m}", tag="ps")
                    for m in range(5)]
            mmsgb = [[] for _ in range(5)]
            for c in range(32):
                if c % 2 == 0:
                    w0t = wp.tile([128, 1280], F16, name="w0tb", tag="w0", bufs=3)
                    nc.sync.dma_start(w0t[:], io["w0gb"][c // 2])
                base = 640 * (c % 2)
                for m in range(5):
                    mmsgb[m].append(nc.tensor.matmul(
                        psgb[m][:], w0t[:, base + 128 * m:base + 128 * m + 128],
                        xtile(c), start=(c == 0), stop=(c == 31)))

            prereduce(2)
            bn_collective(2)

            # ================= under coll2: s0g sweep B + s3 heads =================
            st2h = wp.tile([128, 4 * 512], F16, name="st2h", tag="sel2", bufs=1)
            w1ah = wp.tile([128, 4 * 1536], F16, name="w1ah", tag="w1a", bufs=1)
            for n in range(4):
                nc.scalar.dma_start(st2h[:, 512 * n:512 * (n + 1)], io["sel2"][n])
                nc.sync.dma_start(w1ah[:, 1536 * n:1536 * (n + 1)], io["w1a"][n])

            psC = pp.tile([128, BS], F32, name="psC", tag="psC", bufs=1)
            mmsC = []
            for j in range(16):  # s3 heads from h2
                mmsC.append(nc.tensor.matmul(
                    psC[:], st2h[:, 512 * (j // 4) + 128 * (j % 4):
                                 512 * (j // 4) + 128 * (j % 4) + 128],
                    htile(2, j),
                    start=(j == 0), stop=False))
            for m in range(5):
                chain(mmsgb[m])
                nc.scalar.copy(h0gene[:, BS * (5 + m):BS * (6 + m)], psgb[m][:])

            postmath(2)

            # ================= gap2: act2 + s1 act =================
            for j in range(16):
                tanh_tile(2, j, htile(2, j))
            ps1a = {}
            mms1a = {}
            for j in range(4):
                for (tl, rb, w) in S1_PIECES[j]:
                    if tl not in ps1a:
                        ps1a[tl] = pp.tile([128, BS], F32, name=f"ps1a{tl}", tag="ps")
                        mms1a[tl] = []
                for c in range(4):
                    for (tl, rb, w) in S1_PIECES[j]:
                        lt = tl - S1_BASE_T[j]
                        mms1a[tl].append(nc.tensor.matmul(
                            ps1a[tl][:],
                            w1ah[:, 1536 * j + 384 * c + 128 * lt:
                                 1536 * j + 384 * c + 128 * lt + 128],
                            acttile(2, 4 * j + c),
                            start=(c == 0 and j == TILE_FIRST_TERM[tl]),
                            stop=False))
                for tl, lt in TILE_LAST_TERM.items():
                    if lt == j and tl in ps1a:
                        mms1a[tl].append(nc.tensor.matmul(
                            ps1a[tl][:], eyesb[:],
                            h1gene[:, BS * tl:BS * (tl + 1)],
                            start=False, stop=True))
                        chain(mms1a[tl])
                        copy_stat(1, tl, ps1a[tl])
                        del ps1a[tl]
            prereduce(1)
            bn_collective(1)

            # ================= under coll1: s2 heads + w0a load =================
            st1 = wp.tile([128, 512], F16, name="st1", tag="sel", bufs=1)
            nc.scalar.dma_start(st1[:], io["sel1"][0])
            st0 = wp.tile([128, 128], F16, name="st0", tag="sel0", bufs=1)
            nc.scalar.dma_start(st0[:], io["sel0"][0])
            hw0t = wp.tile([128, 10], F16, name="hw0t", tag="hw0", bufs=1)
            nc.scalar.dma_start(hw0t[:], io["hw0p"][0])
            for k, tl in enumerate((2, 4, 7, 9)):  # s2 heads from h1
                mmsC.append(nc.tensor.matmul(
                    psC[:], st1[:, 128 * k:128 * k + 128],
                    htile(1, tl),
                    start=False, stop=False))
            postmath(1)

            # ================= gap1: act1 + s0 act (one 10-PSUM pass) =================
            for wave, wname in ((0, "w0aa"), (1, "w0ab")):
                ps0 = [pp.tile([128, BS], F32, name=f"ps0a{wave}{i}", tag="ps")
                       for i in range(5)]
                mms0 = [[] for _ in range(5)]
                for k in range(10):
                    if k % 2 == 0:
                        w0at = wp.tile([128, 1280], F16, name="w0at", tag="w0",
                                       bufs=3)
                        nc.sync.dma_start(w0at[:], io[wname][k // 2])
                    if wave == 0:
                        tanh_tile(1, k, htile(1, k))
                    base = 640 * (k % 2)
                    for i in range(5):
                        mms0[i].append(nc.tensor.matmul(
                            ps0[i][:], w0at[:, base + 128 * i:base + 128 * i + 128],
                            acttile(1, k), start=(k == 0), stop=False))
                for i in range(5):
                    m = 5 * wave + i
                    mms0[i].append(nc.tensor.matmul(
                        ps0[i][:], eyesb[:], h0gene[:, BS * m:BS * (m + 1)],
                        start=False, stop=True))
                    chain(mms0[i])
                    copy_stat(0, m, ps0[i])
            # s1 heads from h0 tile 9
            mmsC.append(nc.tensor.matmul(
                psC[:], st0[:, :], htile(0, 9),
                start=False, stop=True))
            chain(mmsC)
            nc.scalar.copy(outsb[:, 2, :], psC[:])
            prereduce(0)
            bn_collective(0)
            nc.scalar.dma_start(y[2], outsb[:, 2, :])
            postmath(0)

            # ================= tail: act0 + s0 head =================
            psD = pp.tile([128, BS], F32, name="psD", tag="ps")
            mmsD = []
            for c in range(10):
                tanh_tile(0, c, htile(0, c))
                mmsD.append(nc.tensor.matmul(
                    psD[0:1, :], hw0t[:, c:c + 1], acttile(0, c),
                    start=(c == 0), stop=(c == 9)))
            chain(mmsD)
            nc.vector.tensor_copy(outsb[0:1, 3, :], psD[0:1, :])
            nc.scalar.dma_start(y[3], outsb[:, 3, :])

    nc.finalize()
    return nc


def kernel(**inputs):
    global _NC, LAST_RESULTS
    inputs = {k: np.asarray(v) for k, v in inputs.items()}
    packed = _pack(inputs)

    x = inputs["x"].astype(np.float32)
    if _NC is None:
        _NC = _build()

    in_maps = []
    for c in range(N_CORES):
        m = dict(packed)
        xs = x[BS * c:BS * (c + 1), :]                    # [256, 4096]
        xT = np.ascontiguousarray(xs.T.astype(np.float16))  # [4096, 256]
        m["xt"] = _ilv(xT.reshape(32, 128, BS).astype(np.float16), 8)
        in_maps.append(m)

    res = run_bass_kernel_spmd(_NC, in_maps, core_ids=list(range(N_CORES)))
    LAST_RESULTS = res

    hb_row = np.concatenate([inputs["hb4"][:, 0], inputs["hb3"][:, 0],
                             inputs["hb2"][:, 0], inputs["hb1"][:, 0],
                             inputs["hb0"][:, 0]]).astype(np.float32)  # [341]
    out = np.empty((B, 341), np.float32)
    for c in range(N_CORES):
        arr = res.results[c]["y"]  # [4, 128, 256]
        headsT = np.concatenate([arr[0], arr[1], arr[2][:84], arr[3][:1]], 0)  # [341, 256]
        out[BS * c:BS * (c + 1), :] = headsT.T + hb_row[None, :]
    return out


# revision 37
# speedup vs baseline: 1.0064x; 1.0064x over previous
import os
import sys

import numpy as np

sys.path.insert(0, "/opt/trn_rl_repo")

import concourse.bacc as bacc
import concourse.bass as bass
import concourse.mybir as mybir
import concourse.tile as tile
from concourse.bass_utils import run_bass_kernel_spmd

# ----- problem config (hardcoded from spec) -----
B = 2048
NUM_GENES = 4096
N_CORES = 8
BS = B // N_CORES  # 256 batch per core
EPS = 1e-5
# (stratum, n_terms, input_dim, output_dim, genes_per_term)
CFG = [(4, 256, 16, 20, 16), (3, 64, 144, 20, 64), (2, 16, 336, 77, 256),
       (1, 4, 1332, 308, 1024), (0, 1, 5328, 1229, 4096)]

F16 = mybir.dt.float16
F32 = mybir.dt.float32

# padded per-term output slots (includes head columns inside the slot)
SLOT = {4: 32, 3: 32, 2: 128, 1: 320, 0: 1280}
NTILES = {4: 64, 3: 16, 2: 16, 1: 10, 0: 10}  # channel tiles of 128 rows per stratum

LAST_RESULTS = None


# ============================================================
# Host-side packing of weights into PE lhsT tile layouts (fp16)
# ============================================================
def _pack(inp):
    W4, W3, W2, W1, W0 = inp["W4"], inp["W3"], inp["W2"], inp["W1"], inp["W0"]
    hw4, hw3, hw2, hw1, hw0 = inp["hw4"], inp["hw3"], inp["hw2"], inp["hw1"], inp["hw0"]
    p = {}

    # s4: 64 groups of 4 terms, paired: [32, 128, 256]
    w4p = np.zeros((32, 128, 256), np.float32)
    for k in range(32):
        for a in range(2):           # group 2k+a
            g = 2 * k + a
            for t in range(4):       # term within group
                term = 4 * g + t
                w4p[k, 64 * a + 16 * t:64 * a + 16 * t + 16,
                    128 * a + 32 * t:128 * a + 32 * t + 20] = W4[term]
    p["w4p"] = w4p

    # s3 gene: strip per 2-term group, full-M 128 cols of the h3 tile
    w3g = np.zeros((32, 128, 128), np.float32)
    for G in range(32):
        for a in range(2):
            term = 2 * G + a
            w3g[G, 64 * a:64 * a + 64, 32 * (term % 4):32 * (term % 4) + 20] = \
                W3[term, 80:144, :]
    p["w3g"] = w3g

    # s3 act: per term, full-M 128 cols; rows = act4T children tile (4 x 32)
    w3a = np.zeros((64, 128, 128), np.float32)
    for j in range(64):
        base = 32 * (j % 4)
        for c in range(4):
            w3a[j, 32 * c:32 * c + 20, base:base + 20] = W3[j, 20 * c:20 * c + 20, :]
            w3a[j, 32 * c:32 * c + 20, base + 20 + c] = hw4[4 * j + c, :, 0]
    p["w3a"] = w3a

    # s2 gene: [32, 128, 128] (term j chunk c2 at idx 2j+c2)
    w2g = np.zeros((32, 128, 128), np.float32)
    for j in range(16):
        for c2 in range(2):
            w2g[2 * j + c2, :, 0:77] = W2[j, 80 + 128 * c2:80 + 128 * c2 + 128, :]
    p["w2g"] = w2g

    # s2 act: [16, 128, 128]; rows = act3T tile (4 children x 32)
    w2a = np.zeros((16, 128, 128), np.float32)
    for j in range(16):
        for c in range(4):
            w2a[j, 32 * c:32 * c + 20, 0:77] = W2[j, 20 * c:20 * c + 20, :]
            w2a[j, 32 * c:32 * c + 20, 77 + c] = hw3[4 * j + c, :, 0]  # s3 head
    p["w2a"] = w2a

    # s1: term j covers h1 rows 320j..320j+319 across 3 tiles; strips hold
    # 3 full-M 128-col blocks (tile-aligned, zero-padded)
    S1_BASE_T = [0, 2, 5, 7]

    def s1_cols(j, o):
        R = 320 * j + o
        return 128 * (R // 128 - S1_BASE_T[j]) + R % 128

    cols308 = {j: np.array([s1_cols(j, o) for o in range(308)]) for j in range(4)}
    w1g = np.zeros((32, 128, 384), np.float32)
    for j in range(4):
        for c in range(8):
            w1g[8 * j + c][:, cols308[j]] = W1[j, 308 + 128 * c:308 + 128 * c + 128, :]
    p["w1g"] = w1g

    w1a = np.zeros((16, 128, 384), np.float32)
    for j in range(4):
        for c in range(4):
            w1a[4 * j + c][0:77, cols308[j]] = W1[j, 77 * c:77 * c + 77, :]
            w1a[4 * j + c][0:77, s1_cols(j, 308 + c)] = hw2[4 * j + c, :, 0]
    p["w1a"] = w1a

    # s0 gene: split into two 5-out-tile sweep tensors (each streamed once)
    w0g = np.zeros((32, 128, 1280), np.float32)
    for c in range(32):
        w0g[c, :, 0:1229] = W0[0, 1232 + 128 * c:1232 + 128 * c + 128, :]
    p["w0ga"] = np.ascontiguousarray(w0g[:, :, :640])
    p["w0gb"] = np.ascontiguousarray(w0g[:, :, 640:])

    # s0 act: [10, 128, 1280]; rows = act1T (4 terms x 320, 308 real)
    w0a = np.zeros((10, 128, 1280), np.float32)
    for c in range(10):
        for r in range(128):
            R = 128 * c + r
            j, rr = R // 320, R % 320
            if rr < 308:
                w0a[c, r, 0:1229] = W0[0, 308 * j + rr, :]
                w0a[c, r, 1229 + j] = hw1[j, rr, 0]  # s1 head
    p["w0aa"] = np.ascontiguousarray(w0a[:, :, :640])
    p["w0ab"] = np.ascontiguousarray(w0a[:, :, 640:])

    # head-extraction selection matrices
    sel3 = np.zeros((16, 128, 128), np.float32)
    for j2 in range(16):
        for jj in range(4):
            for c in range(4):
                sel3[j2, 32 * jj + 20 + c, 16 * (j2 % 8) + 4 * jj + c] = 1.0
    p["sel3"] = sel3
    sel2 = np.zeros((16, 128, 128), np.float32)
    for j in range(16):
        for c in range(4):
            sel2[j, 77 + c, 4 * j + c] = 1.0
    p["sel2"] = sel2
    sel1 = np.zeros((4, 128, 128), np.float32)
    rowk = [52, 116, 52, 116]
    for k in range(4):
        for c in range(4):
            sel1[k, rowk[k] + c, 64 + 4 * k + c] = 1.0
    p["sel1"] = sel1
    sel0 = np.zeros((1, 128, 128), np.float32)
    for c in range(4):
        sel0[0, 77 + c, 80 + c] = 1.0
    p["sel0"] = sel0

    hw0p = np.zeros((10, 128, 1), np.float32)
    for c in range(10):
        n = min(128, 1229 - 128 * c)
        if n > 0:
            hw0p[c, :n, 0] = hw0[0, 128 * c:128 * c + n, 0]
    p["hw0p"] = hw0p

    p["eye"] = np.eye(128, dtype=np.float32)[None]

    out16 = {}
    for k, v in p.items():
        v16 = v.astype(np.float16)
        if k in _ILV_N:
            v16 = _ilv(v16, _ILV_N[k])
        out16[k] = np.ascontiguousarray(v16)

    # g / bb channel-tiled f32 vectors [128, ntiles] (pad rows -> 0)
    def tile_vec(vec_f, s):
        nt = NTILES[s]
        slot = SLOT[s]
        out = np.zeros((128, nt), np.float32)
        for T in range(nt):
            for prt in range(128):
                R = 128 * T + prt
                j, o = R // slot, R % slot
                v = vec_f(j, o)
                if v is not None:
                    out[prt, T] = v
        return out

    gts, bbs = [], []
    for s, T_, I_, O_, _ in CFG:
        g, bb = inp[f"g{s}"], inp[f"bb{s}"]
        gts.append(tile_vec(lambda j, o: g[j, o] if (j < T_ and o < O_) else None, s))
        bbs.append(tile_vec(lambda j, o: bb[j, o] if (j < T_ and o < O_) else None, s))
    out16["gtall"] = np.ascontiguousarray(np.concatenate(gts, axis=1))
    out16["bball"] = np.ascontiguousarray(np.concatenate(bbs, axis=1))
    return out16


def _ilv(arr, n):
    """[nb, 128, F] -> [nb//n, 128, n*F]: n strips side-by-side per partition
    (matches the SBUF tile layout, so DMAs need no rearrange and get one
    contiguous chunk per partition)."""
    nb, p, f = arr.shape
    assert nb % n == 0
    return np.ascontiguousarray(
        arr.reshape(nb // n, n, p, f).transpose(0, 2, 1, 3).reshape(nb // n, p, n * f))

_ILV_N = {"w4p": 4, "w3g": 4, "w3a": 8, "w2g": 4, "w2a": 4, "w1g": 8,
          "w1a": 4, "w0ga": 2, "w0gb": 2, "w0aa": 2, "w0ab": 2, "sel3": 4, "sel2": 4,
          "sel1": 4, "hw0p": 10}


# ============================================================
# Bass program (built once, shared across calls)
# ============================================================
_NC = None

# s1 out-piece map: term j -> list of (tile, row_base, width) covering rows 320j..320j+319
S1_PIECES = {
    0: [(0, 0, 128), (1, 0, 128), (2, 0, 64)],
    1: [(2, 64, 64), (3, 0, 128), (4, 0, 128)],
    2: [(5, 0, 128), (6, 0, 128), (7, 0, 64)],
    3: [(7, 64, 64), (8, 0, 128), (9, 0, 128)],
}
TILE_LAST_TERM = {0: 0, 1: 0, 2: 1, 3: 1, 4: 1, 5: 2, 6: 2, 7: 3, 8: 3, 9: 3}
TILE_FIRST_TERM = {0: 0, 1: 0, 2: 0, 3: 1, 4: 1, 5: 2, 6: 2, 7: 2, 8: 3, 9: 3}
S1_BASE_T = [0, 2, 5, 7]


def _build():
    nc = bacc.Bacc("TRN2", target_bir_lowering=False, debug=False,
                   enable_asserts=True, num_devices=N_CORES)
    dep = bass._add_dep_helper
    io = {}
    io["xt"] = nc.dram_tensor("xt", [4, 128, 8 * BS], F16, kind="ExternalInput")
    for name, shp in [("w4p", [8, 128, 1024]), ("w3g", [8, 128, 512]),
                      ("w3a", [8, 128, 1024]), ("w2g", [8, 128, 512]),
                      ("w2a", [4, 128, 512]), ("w1g", [4, 128, 3072]),
                      ("w1a", [4, 128, 1536]), ("w0ga", [16, 128, 1280]),
                      ("w0gb", [16, 128, 1280]), ("w0aa", [5, 128, 1280]),
                      ("w0ab", [5, 128, 1280]),
                      ("sel3", [4, 128, 512]), ("sel2", [4, 128, 512]),
                      ("sel1", [1, 128, 512]), ("sel0", [1, 128, 128]),
                      ("hw0p", [1, 128, 10]), ("eye", [1, 128, 128])]:
        io[name] = nc.dram_tensor(name, shp, F16, kind="ExternalInput")
    NTOT = sum(NTILES.values())
    io["gtall"] = nc.dram_tensor("gtall", [128, NTOT], F32, kind="ExternalInput")
    io["bball"] = nc.dram_tensor("bball", [128, NTOT], F32, kind="ExternalInput")
    y = nc.dram_tensor("y", [4, 128, BS], F32, kind="ExternalOutput")

    rg = [list(range(N_CORES))]

    with tile.TileContext(nc, num_cores=N_CORES) as tc:
        with tc.tile_pool(name="per", bufs=1) as per, \
             tc.tile_pool(name="wp", bufs=3) as wp, \
             tc.tile_pool(name="pp", bufs=7, space="PSUM") as pp, \
             tc.tile_pool(name="dp", bufs=1, space="DRAM") as dp:

            # ---- persistent SBUF ----
            xsb = per.tile([128, 32 * BS], F16, name="xsb", tag="xsb")
            actT = {s: per.tile([128, NTILES[s] * BS], F16, name=f"act{s}", tag=f"act{s}")
                    for s in (4, 3, 2, 1, 0)}
            hT = {s: per.tile([128, NTILES[s] * BS], F16, name=f"h{s}", tag=f"h{s}")
                  for s in (3, 2, 1, 0)}
            h1gene = per.tile([128, 10 * BS], F16, name="h1gene", tag="h1gene")
            h0gene = per.tile([128, 10 * BS], F16, name="h0gene", tag="h0gene")
            stats = {s: per.tile([128, NTILES[s] * 6], F32, name=f"st{s}", tag=f"st{s}")
                     for s in (4, 3, 2, 1, 0)}
            agg = per.tile([128, 8 * NTILES[4] * 2], F32, name="agg", tag="agg")
            ccs = per.tile([128, NTILES[4] * 2], F32, name="ccs", tag="ccs")
            prtmp = per.tile([128, NTILES[4] * 2], F32, name="prtmp", tag="prtmp")
            prtmp2 = per.tile([128, NTILES[4]], F32, name="prtmp2", tag="prtmp2")
            aT = {s: per.tile([128, NTILES[s]], F32, name=f"aT{s}", tag=f"aT{s}")
                  for s in (4, 3, 2, 1, 0)}
            cT = {s: per.tile([128, NTILES[s]], F32, name=f"cT{s}", tag=f"cT{s}")
                  for s in (4, 3, 2, 1, 0)}
            sd_t = {s: per.tile([128, NTILES[s]], F32, name=f"sd{s}", tag=f"sd{s}")
                    for s in (4, 3, 2, 1, 0)}
            mub = per.tile([128, NTILES[4]], F32, name="mub", tag="mub")
            varb = per.tile([128, NTILES[4]], F32, name="varb", tag="varb")
            gsb_all = per.tile([128, 116], F32, name="gsb_all", tag="gsb_all")
            bbsb_all = per.tile([128, 116], F32, name="bbsb_all", tag="bbsb_all")
            _off = {}
            _o = 0
            for s in (4, 3, 2, 1, 0):
                _off[s] = _o
                _o += NTILES[s]
            gsb = {s: gsb_all[:, _off[s]:_off[s] + NTILES[s]] for s in (4, 3, 2, 1, 0)}
            bbsb = {s: bbsb_all[:, _off[s]:_off[s] + NTILES[s]] for s in (4, 3, 2, 1, 0)}
            outsb = per.tile([128, 4, BS], F32, name="outsb", tag="outsb")
            zbuf = per.tile([128, 8 * BS], F16, name="zbuf", tag="zbuf")
            eps_sb = per.tile([128, 1], F32, name="eps_sb", tag="eps_sb")
            eyesb = per.tile([128, 128], F16, name="eyesb", tag="eyesb")
            nc.vector.memset(eps_sb[:], EPS)
            nc.vector.memset(outsb[:, 3, :], 0.0)

            # ---- front DMAs: xt spread over 3 queues (ACT kept for w4t) ----
            nc.sync.dma_start(xsb[:, 0:8 * BS], io["xt"][0])
            nc.gpsimd.dma_start(xsb[:, 8 * BS:16 * BS], io["xt"][1])
            nc.gpsimd.dma_start(xsb[:, 16 * BS:24 * BS], io["xt"][2])
            nc.gpsimd.dma_start(xsb[:, 24 * BS:32 * BS], io["xt"][3])

            def xtile(t):
                return xsb[:, BS * t:BS * (t + 1)]

            def acttile(s, t):
                return actT[s][:, BS * t:BS * (t + 1)]

            def htile(s, t):
                return hT[s][:, BS * t:BS * (t + 1)]

            def chain(mms):
                for a, b in zip(mms, mms[1:]):
                    dep(b.ins, a.ins, sync=False,
                        reason="psum accumulation order")

            # slim stats prereduce + collective + postmath
            # stats layout: one 6-field record per tile, halves of 128:
            # (c0, m0, M2_0, c1, m1, M2_1)
            def prereduce(s, lo=0, n=None, coff=0):
                n = NTILES[s] if n is None else n
                sv = stats[s][:, 6 * lo:6 * (lo + n)].rearrange(
                    "p (t e th) -> p t e th", e=2, th=3)
                mv = sv[:, :, :, 1]   # per-half means (128 samples) [128, n, 2]
                vv = sv[:, :, :, 2]   # per-half M2 sums             [128, n, 2]
                msq = prtmp[:, :2 * n].rearrange("p (t e) -> p t e", e=2)
                f0 = ccs[:, coff:coff + n]
                f1 = ccs[:, coff + n:coff + 2 * n]
                nc.vector.tensor_tensor(msq, mv, mv, op=mybir.AluOpType.mult)
                nc.vector.tensor_tensor(f0, mv[:, :, 0], mv[:, :, 1],
                                        op=mybir.AluOpType.add)
                nc.vector.tensor_tensor(prtmp2[:, :n], msq[:, :, 0], msq[:, :, 1],
                                        op=mybir.AluOpType.add)
                nc.vector.tensor_scalar_mul(prtmp2[:, :n], prtmp2[:, :n], 128.0)
                nc.vector.tensor_tensor(f1, vv[:, :, 0], vv[:, :, 1],
                                        op=mybir.AluOpType.add)
                nc.vector.tensor_tensor(f1, f1, prtmp2[:, :n],
                                        op=mybir.AluOpType.add)

            def bn_collective(s, n=None, coff=0, aoff=0, suf=""):
                n = NTILES[s] if n is None else n
                F = 2 * n
                cc_in = dp.tile([128, F], F32, name=f"cci{s}{suf}")
                cc_out = dp.tile([N_CORES, 128, F], F32, name=f"cco{s}{suf}",
                                 addr_space="Shared")
                nc.gpsimd.dma_start(cc_in[:], ccs[:, coff:coff + F])
                nc.gpsimd.collective_compute(
                    "AllGather", mybir.AluOpType.bypass, replica_groups=rg,
                    ins=[cc_in.opt()], outs=[cc_out.opt()])
                nc.gpsimd.dma_start(agg[:, aoff:aoff + 8 * F],
                                    cc_out.rearrange("c p f -> p c f"))

            def postmath(s, lo=0, n=None, aoff=0):
                n = NTILES[s] if n is None else n
                F = 2 * n
                ag = agg[:, aoff:aoff + 8 * F]
                nc.vector.tensor_tensor(ag[:, 0:4 * F], ag[:, 0:4 * F],
                                        ag[:, 4 * F:8 * F], op=mybir.AluOpType.add)
                nc.vector.tensor_tensor(ag[:, 0:2 * F], ag[:, 0:2 * F],
                                        ag[:, 2 * F:4 * F], op=mybir.AluOpType.add)
                nc.vector.tensor_tensor(ag[:, 0:F], ag[:, 0:F],
                                        ag[:, F:2 * F], op=mybir.AluOpType.add)
                mu = mub[:, lo:lo + n]
                va = varb[:, lo:lo + n]
                sd = sd_t[s][:, lo:lo + n]
                nc.vector.tensor_scalar_mul(mu, ag[:, 0:n], 1.0 / 16)
                nc.vector.tensor_scalar_mul(va, ag[:, n:F], 1.0 / 2048)
                nc.vector.tensor_tensor(prtmp2[:, :n], mu, mu,
                                        op=mybir.AluOpType.mult)
                nc.vector.tensor_tensor(va, va, prtmp2[:, :n],
                                        op=mybir.AluOpType.subtract)
                nc.scalar.activation(sd, va,
                                     mybir.ActivationFunctionType.Sqrt,
                                     bias=eps_sb[:, 0:1])
                nc.vector.reciprocal(sd, sd)
                nc.vector.tensor_tensor(aT[s][:, lo:lo + n], sd,
                                        gsb[s][:, lo:lo + n],
                                        op=mybir.AluOpType.mult)
                nc.vector.tensor_tensor(sd, mu, aT[s][:, lo:lo + n],
                                        op=mybir.AluOpType.mult)
                nc.vector.tensor_tensor(cT[s][:, lo:lo + n],
                                        bbsb[s][:, lo:lo + n], sd,
                                        op=mybir.AluOpType.subtract)

            def tanh_tile(s, t, src):
                nc.scalar.activation(
                    acttile(s, t), src,
                    mybir.ActivationFunctionType.Tanh,
                    bias=cT[s][:, t:t + 1], scale=aT[s][:, t:t + 1])

            def copy_stat(s, t, ps):
                # h tile: PSUM f32 -> SBUF fp16 (ACT/DVE alternate; gpsimd
                # cannot read PSUM), per-tile fp16 bn_stats on DVE
                if t % 2 == 0:
                    nc.scalar.copy(htile(s, t), ps[:])
                else:
                    nc.vector.tensor_copy(htile(s, t), ps[:])
                nc.vector.bn_stats(stats[s][:, 6 * t:6 * t + 6], htile(s, t))

            # ================= s4 pass 1: stats only =================
            def s4_mm(t, ps, wt):
                a = t % 2
                rhs = xsb[64 * a:64 * a + 64, BS * (t // 2):BS * (t // 2) + BS]
                return nc.tensor.matmul(
                    ps[:], wt[64 * a:64 * a + 64, 128 * a:128 * a + 128],
                    rhs, start=True, stop=True)

            w4keep = {}

            def s4p1_range(P0, P1):
                for P in range(P0, P1):
                    if P % 4 == 0:
                        w4t = wp.tile([128, 1024], F16, name="w4t", tag="w4",
                                      bufs=8)
                        nc.sync.dma_start(w4t[:], io["w4p"][P // 4])
                        w4keep[P // 4] = w4t
                    w4v = w4t[:, 256 * (P % 4):256 * (P % 4) + 256]
                    for a in range(2):
                        t = 2 * P + a
                        ps = pp.tile([128, BS], F32, name="ps4", tag="ps")
                        s4_mm(t, ps, w4v)
                        if t % 4 == 0:
                            # direct f32 stats from PSUM on DVE (no copy)
                            nc.vector.bn_stats(stats[4][:, 6 * t:6 * t + 6],
                                               ps[:])
                        else:
                            sc = acttile(4, t)
                            nc.scalar.copy(sc, ps[:])
                            nc.vector.bn_stats(stats[4][:, 6 * t:6 * t + 6], sc)

            # s4 stats in two halves; half-A collective fires at mid-front so
            # half-A tanh + s3-A overlap the half-B collective
            s4p1_range(0, 16)
            nc.sync.dma_start(gsb_all[:], io["gtall"][:])
            nc.sync.dma_start(bbsb_all[:], io["bball"][:])
            nc.sync.dma_start(eyesb[:], io["eye"][0])
            prereduce(4, lo=0, n=32, coff=0)
            bn_collective(4, n=32, coff=0, aoff=0, suf="a")
            s4p1_range(16, 32)
            prereduce(4, lo=32, n=32, coff=64)
            bn_collective(4, n=32, coff=64, aoff=512, suf="b")

            # ================= s1 gene (under coll4) =================
            ps1 = {}
            mms1 = {}
            for j in range(4):
                for (tl, rb, w) in S1_PIECES[j]:
                    if tl not in ps1:
                        ps1[tl] = pp.tile([128, BS], F32, name=f"ps1g{tl}", tag="ps")
                        mms1[tl] = []
                w1t = wp.tile([128, 8 * 384], F16, name="w1t", tag="w1", bufs=2)
                nc.sync.dma_start(w1t[:], io["w1g"][j])
                for c in range(8):
                    for (tl, rb, w) in S1_PIECES[j]:
                        lt = tl - S1_BASE_T[j]
                        mms1[tl].append(nc.tensor.matmul(
                            ps1[tl][:], w1t[:, 384 * c + 128 * lt:384 * c + 128 * lt + 128],
                            xtile(8 * j + c),
                            start=(c == 0 and j == TILE_FIRST_TERM[tl]),
                            stop=(c == 7 and j == TILE_LAST_TERM[tl])))
                for tl, lt in TILE_LAST_TERM.items():
                    if lt == j and tl in ps1:
                        chain(mms1[tl])
                        if tl % 2 == 0:
                            nc.scalar.copy(h1gene[:, BS * tl:BS * (tl + 1)],
                                           ps1[tl][:])
                        else:
                            nc.vector.tensor_copy(
                                h1gene[:, BS * tl:BS * (tl + 1)], ps1[tl][:])
                        del ps1[tl]


            # ================= gap4: s4 pass 2 + s3 interleaved =================
            # (w4p pass 2 / w3g / w3a stream on the SP queue, idle in this phase)
            def gap4_range(P0, P1):
              for P in range(P0, P1):
                w4t2 = w4keep[P // 4]
                w4v2 = w4t2[:, 256 * (P % 4):256 * (P % 4) + 256]
                for a in range(2):
                    t = 2 * P + a
                    ps = pp.tile([128, BS], F32, name="ps4b", tag="ps")
                    s4_mm(t, ps, w4v2)
                    if P % 2 == 0:
                        # first half of each 4-tile group: DVE prescale into
                        # zbuf (fast PSUM release), wide plain tanh later
                        zs = (t // 4 % 2) * 2 + t % 4
                        zslot = zbuf[:, BS * zs:BS * (zs + 1)]
                        nc.vector.tensor_scalar(zslot, ps[:], aT[4][:, t:t + 1],
                                                cT[4][:, t:t + 1],
                                                op0=mybir.AluOpType.mult,
                                                op1=mybir.AluOpType.add)
                    else:
                        # second half: fused scale/bias tanh from PSUM on ACT
                        tanh_tile(4, t, ps[:])
                if P % 2 == 1:
                    t3 = P // 2
                    # plain wide tanh over the 2 prescaled tiles 4*t3, 4*t3+1
                    h = ((t3 % 2) * 2) * BS
                    nc.scalar.activation(
                        actT[4][:, BS * 4 * t3:BS * (4 * t3 + 2)],
                        zbuf[:, h:h + 2 * BS],
                        mybir.ActivationFunctionType.Tanh)
                    if t3 % 2 == 0:
                        w3at = wp.tile([128, 1024], F16, name="w3at", tag="w3a", bufs=4)
                        nc.sync.dma_start(w3at[:], io["w3a"][t3 // 2])
                        w3gt = wp.tile([128, 512], F16, name="w3gt", tag="w3g", bufs=4)
                        nc.sync.dma_start(w3gt[:], io["w3g"][t3 // 2])
                    ps = pp.tile([128, BS], F32, name="ps3", tag="ps")
                    mms = []
                    for a in range(2):  # gene groups (full-M padded)
                        G = 2 * t3 + a
                        goff = 128 * (G % 4)
                        mms.append(nc.tensor.matmul(
                            ps[:], w3gt[:, goff:goff + 128], xtile(G),
                            start=(a == 0), stop=False))
                    for jj in range(4):  # act terms (full-M padded)
                        j = 4 * t3 + jj
                        k = 128 * (j % 8)
                        mms.append(nc.tensor.matmul(
                            ps[:], w3at[:, k:k + 128],
                            acttile(4, j), start=False, stop=(jj == 3)))
                    chain(mms)
                    copy_stat(3, t3, ps)
            postmath(4, lo=0, n=32, aoff=0)
            gap4_range(0, 16)
            postmath(4, lo=32, n=32, aoff=512)
            gap4_range(16, 32)

            prereduce(3)
            bn_collective(3)

            # ================= under coll3: s4 heads + s0g sweep A =================
            w2ah = wp.tile([128, 4 * 512], F16, name="w2ah", tag="w2a", bufs=1)
            st3h = wp.tile([128, 4 * 512], F16, name="st3h", tag="sel3", bufs=1)
            for n in range(4):
                nc.scalar.dma_start(w2ah[:, 512 * n:512 * (n + 1)], io["w2a"][n])
                nc.scalar.dma_start(st3h[:, 512 * n:512 * (n + 1)], io["sel3"][n])

            psA = pp.tile([128, BS], F32, name="psA", tag="ps")
            psB = pp.tile([128, BS], F32, name="psB", tag="ps")
            mmsA, mmsB = [], []
            for j2 in range(16):
                mm = nc.tensor.matmul(
                    (psA if j2 < 8 else psB)[:],
                    st3h[:, 512 * (j2 // 4) + 128 * (j2 % 4):
                         512 * (j2 // 4) + 128 * (j2 % 4) + 128],
                    htile(3, j2),
                    start=(j2 % 8 == 0), stop=(j2 % 8 == 7))
                (mmsA if j2 < 8 else mmsB).append(mm)
            chain(mmsA)
            chain(mmsB)
            nc.scalar.copy(outsb[:, 0, :], psA[:])
            nc.scalar.copy(outsb[:, 1, :], psB[:])
            nc.scalar.dma_start(y[0], outsb[:, 0, :])
            nc.scalar.dma_start(y[1], outsb[:, 1, :])

            # s0g sweep A: fills the coll3 window
            psga = [pp.tile([128, BS], F32, name=f"ps0ga{m}", tag="ps")
                    for m in range(5)]
            mmsga = [[] for _ in range(5)]
            for c in range(32):
                if c % 2 == 0:
                    w0t = wp.tile([128, 1280], F16, name="w0t", tag="w0", bufs=3)
                    nc.sync.dma_start(w0t[:], io["w0ga"][c // 2])
                base = 640 * (c % 2)
                for m in range(5):
                    mmsga[m].append(nc.tensor.matmul(
                        psga[m][:], w0t[:, base + 128 * m:base + 128 * m + 128],
                        xtile(c), start=(c == 0), stop=(c == 31)))
            for m in range(5):
                chain(mmsga[m])
                nc.scalar.copy(h0gene[:, BS * m:BS * (m + 1)], psga[m][:])

            postmath(3)

            # ================= gap3: act3 + s2 =================
            for j in range(16):
                if j % 2 == 0:
                    w2gt = wp.tile([128, 512], F16, name="w2gt", tag="w2g", bufs=2)
                    nc.gpsimd.dma_start(w2gt[:], io["w2g"][j // 2])
                tanh_tile(3, j, htile(3, j))
                ps = pp.tile([128, BS], F32, name="ps2", tag="ps")
                mms = []
                for c2 in range(2):
                    goff = 128 * ((2 * j + c2) % 4)
                    mms.append(nc.tensor.matmul(
                        ps[:], w2gt[:, goff:goff + 128], xtile(2 * j + c2),
                        start=(c2 == 0), stop=False))
                aoff = 512 * (j // 4) + 128 * (j % 4)
                mms.append(nc.tensor.matmul(
                    ps[:], w2ah[:, aoff:aoff + 128], acttile(3, j),
                    start=False, stop=True))
                chain(mms)
                copy_stat(2, j, ps)
            # s0g sweep B: fills the cci2/coll2 window
            psgb = [pp.tile([128, BS], F32, name=f"ps0gb{m}", tag="ps")
                    for m in range(5)]
            mmsgb = [[] for _ in range(5)]
            for c in range(32):
                if c % 2 == 0:
                    w0t = wp.tile([128, 1280], F16, name="w0tb", tag="w0", bufs=3)
                    nc.sync.dma_start(w0t[:], io["w0gb"][c // 2])
                base = 640 * (c % 2)
                for m in range(5):
                    mmsgb[m].append(nc.tensor.matmul(
                        psgb[m][:], w0t[:, base + 128 * m:base + 128 * m + 128],
                        xtile(c), start=(c == 0), stop=(c == 31)))

            prereduce(2)
            bn_collective(2)

            # ================= under coll2: s0g sweep B + s3 heads =================
            st2h = wp.tile([128, 4 * 512], F16, name="st2h", tag="sel2", bufs=1)
            w1ah = wp.tile([128, 4 * 1536], F16, name="w1ah", tag="w1a", bufs=1)
            for n in range(4):
                nc.scalar.dma_start(st2h[:, 512 * n:512 * (n + 1)], io["sel2"][n])
                nc.sync.dma_start(w1ah[:, 1536 * n:1536 * (n + 1)], io["w1a"][n])

            psC = pp.tile([128, BS], F32, name="psC", tag="psC", bufs=1)
            mmsC = []
            for j in range(16):  # s3 heads from h2
                mmsC.append(nc.tensor.matmul(
                    psC[:], st2h[:, 512 * (j // 4) + 128 * (j % 4):
                                 512 * (j // 4) + 128 * (j % 4) + 128],
                    htile(2, j),
                    start=(j == 0), stop=False))
            for m in range(5):
                chain(mmsgb[m])
                nc.scalar.copy(h0gene[:, BS * (5 + m):BS * (6 + m)], psgb[m][:])

            postmath(2)

            # ================= gap2: act2 + s1 act =================
            for j in range(16):
                tanh_tile(2, j, htile(2, j))
            ps1a = {}
            mms1a = {}
            for j in range(4):
                for (tl, rb, w) in S1_PIECES[j]:
                    if tl not in ps1a:
                        ps1a[tl] = pp.tile([128, BS], F32, name=f"ps1a{tl}", tag="ps")
                        mms1a[tl] = []
                for c in range(4):
                    for (tl, rb, w) in S1_PIECES[j]:
                        lt = tl - S1_BASE_T[j]
                        mms1a[tl].append(nc.tensor.matmul(
                            ps1a[tl][:],
                            w1ah[:, 1536 * j + 384 * c + 128 * lt:
                                 1536 * j + 384 * c + 128 * lt + 128],
                            acttile(2, 4 * j + c),
                            start=(c == 0 and j == TILE_FIRST_TERM[tl]),
                            stop=False))
                for tl, lt in TILE_LAST_TERM.items():
                    if lt == j and tl in ps1a:
                        mms1a[tl].append(nc.tensor.matmul(
                            ps1a[tl][:], eyesb[:],
                            h1gene[:, BS * tl:BS * (tl + 1)],
                            start=False, stop=True))
                        chain(mms1a[tl])
                        copy_stat(1, tl, ps1a[tl])
                        del ps1a[tl]
            prereduce(1)
            bn_collective(1)

            # ================= under coll1: s2 heads + w0a load =================
            st1 = wp.tile([128, 512], F16, name="st1", tag="sel", bufs=1)
            nc.scalar.dma_start(st1[:], io["sel1"][0])
            st0 = wp.tile([128, 128], F16, name="st0", tag="sel0", bufs=1)
            nc.scalar.dma_start(st0[:], io["sel0"][0])
            hw0t = wp.tile([128, 10], F16, name="hw0t", tag="hw0", bufs=1)
            nc.scalar.dma_start(hw0t[:], io["hw0p"][0])
            for k, tl in enumerate((2, 4, 7, 9)):  # s2 heads from h1
                mmsC.append(nc.tensor.matmul(
                    psC[:], st1[:, 128 * k:128 * k + 128],
                    htile(1, tl),
                    start=False, stop=False))
            postmath(1)

            # ================= gap1: act1 + s0 act (one 10-PSUM pass) =================
            for wave, wname in ((0, "w0aa"), (1, "w0ab")):
                ps0 = [pp.tile([128, BS], F32, name=f"ps0a{wave}{i}", tag="ps")
                       for i in range(5)]
                mms0 = [[] for _ in range(5)]
                for k in range(10):
                    if k % 2 == 0:
                        w0at = wp.tile([128, 1280], F16, name="w0at", tag="w0",
                                       bufs=3)
                        nc.sync.dma_start(w0at[:], io[wname][k // 2])
                    if wave == 0:
                        tanh_tile(1, k, htile(1, k))
                    base = 640 * (k % 2)
                    for i in range(5):
                        mms0[i].append(nc.tensor.matmul(
                            ps0[i][:], w0at[:, base + 128 * i:base + 128 * i + 128],
                            acttile(1, k), start=(k == 0), stop=False))
                for i in range(5):
                    m = 5 * wave + i
                    mms0[i].append(nc.tensor.matmul(
                        ps0[i][:], eyesb[:], h0gene[:, BS * m:BS * (m + 1)],
                        start=False, stop=True))
                    chain(mms0[i])
                    copy_stat(0, m, ps0[i])
            # s1 heads from h0 tile 9
            mmsC.append(nc.tensor.matmul(
                psC[:], st0[:, :], htile(0, 9),
                start=False, stop=True))
            chain(mmsC)
            nc.scalar.copy(outsb[:, 2, :], psC[:])
            prereduce(0)
            bn_collective(0)
            nc.scalar.dma_start(y[2], outsb[:, 2, :])
            postmath(0)

            # ================= tail: act0 + s0 head =================
            psD = pp.tile([128, BS], F32, name="psD", tag="ps")
            mmsD = []
            for c in range(10):
                tanh_tile(0, c, htile(0, c))
                mmsD.append(nc.tensor.matmul(
                    psD[0:1, :], hw0t[:, c:c + 1], acttile(0, c),
                    start=(c == 0), stop=(c == 9)))
            chain(mmsD)
            nc.vector.tensor_copy(outsb[0:1, 3, :], psD[0:1, :])
            nc.scalar.dma_start(y[3], outsb[:, 3, :])

    nc.finalize()
    return nc


def kernel(**inputs):
    global _NC, LAST_RESULTS
    inputs = {k: np.asarray(v) for k, v in inputs.items()}
    packed = _pack(inputs)

    x = inputs["x"].astype(np.float32)
    if _NC is None:
        _NC = _build()

    in_maps = []
    for c in range(N_CORES):
        m = dict(packed)
        xs = x[BS * c:BS * (c + 1), :]                    # [256, 4096]
        xT = np.ascontiguousarray(xs.T.astype(np.float16))  # [4096, 256]
        m["xt"] = _ilv(xT.reshape(32, 128, BS).astype(np.float16), 8)
        in_maps.append(m)

    res = run_bass_kernel_spmd(_NC, in_maps, core_ids=list(range(N_CORES)))
    LAST_RESULTS = res

    hb_row = np.concatenate([inputs["hb4"][:, 0], inputs["hb3"][:, 0],
                             inputs["hb2"][:, 0], inputs["hb1"][:, 0],
                             inputs["hb0"][:, 0]]).astype(np.float32)  # [341]
    out = np.empty((B, 341), np.float32)
    for c in range(N_CORES):
        arr = res.results[c]["y"]  # [4, 128, 256]
        headsT = np.concatenate([arr[0], arr[1], arr[2][:84], arr[3][:1]], 0)  # [341, 256]
        out[BS * c:BS * (c + 1), :] = headsT.T + hb_row[None, :]
    return out


# revision 38
# speedup vs baseline: 1.0113x; 1.0048x over previous
import os
import sys

import numpy as np

sys.path.insert(0, "/opt/trn_rl_repo")

import concourse.bacc as bacc
import concourse.bass as bass
import concourse.mybir as mybir
import concourse.tile as tile
from concourse.bass_utils import run_bass_kernel_spmd

# ----- problem config (hardcoded from spec) -----
B = 2048
NUM_GENES = 4096
N_CORES = 8
BS = B // N_CORES  # 256 batch per core
EPS = 1e-5
# (stratum, n_terms, input_dim, output_dim, genes_per_term)
CFG = [(4, 256, 16, 20, 16), (3, 64, 144, 20, 64), (2, 16, 336, 77, 256),
       (1, 4, 1332, 308, 1024), (0, 1, 5328, 1229, 4096)]

F16 = mybir.dt.float16
F32 = mybir.dt.float32

# padded per-term output slots (includes head columns inside the slot)
SLOT = {4: 32, 3: 32, 2: 128, 1: 320, 0: 1280}
NTILES = {4: 64, 3: 16, 2: 16, 1: 10, 0: 10}  # channel tiles of 128 rows per stratum

LAST_RESULTS = None


# ============================================================
# Host-side packing of weights into PE lhsT tile layouts (fp16)
# ============================================================
def _pack(inp):
    W4, W3, W2, W1, W0 = inp["W4"], inp["W3"], inp["W2"], inp["W1"], inp["W0"]
    hw4, hw3, hw2, hw1, hw0 = inp["hw4"], inp["hw3"], inp["hw2"], inp["hw1"], inp["hw0"]
    p = {}

    # s4: 64 groups of 4 terms, paired: [32, 128, 256]
    w4p = np.zeros((32, 128, 256), np.float32)
    for k in range(32):
        for a in range(2):           # group 2k+a
            g = 2 * k + a
            for t in range(4):       # term within group
                term = 4 * g + t
                w4p[k, 64 * a + 16 * t:64 * a + 16 * t + 16,
                    128 * a + 32 * t:128 * a + 32 * t + 20] = W4[term]
    p["w4p"] = w4p

    # s3 gene: strip per 2-term group, full-M 128 cols of the h3 tile
    w3g = np.zeros((32, 128, 128), np.float32)
    for G in range(32):
        for a in range(2):
            term = 2 * G + a
            w3g[G, 64 * a:64 * a + 64, 32 * (term % 4):32 * (term % 4) + 20] = \
                W3[term, 80:144, :]
    p["w3g"] = w3g

    # s3 act: per term, full-M 128 cols; rows = act4T children tile (4 x 32)
    w3a = np.zeros((64, 128, 128), np.float32)
    for j in range(64):
        base = 32 * (j % 4)
        for c in range(4):
            w3a[j, 32 * c:32 * c + 20, base:base + 20] = W3[j, 20 * c:20 * c + 20, :]
            w3a[j, 32 * c:32 * c + 20, base + 20 + c] = hw4[4 * j + c, :, 0]
    p["w3a"] = w3a

    # s2 gene: [32, 128, 128] (term j chunk c2 at idx 2j+c2)
    w2g = np.zeros((32, 128, 128), np.float32)
    for j in range(16):
        for c2 in range(2):
            w2g[2 * j + c2, :, 0:77] = W2[j, 80 + 128 * c2:80 + 128 * c2 + 128, :]
    p["w2g"] = w2g

    # s2 act: [16, 128, 128]; rows = act3T tile (4 children x 32)
    w2a = np.zeros((16, 128, 128), np.float32)
    for j in range(16):
        for c in range(4):
            w2a[j, 32 * c:32 * c + 20, 0:77] = W2[j, 20 * c:20 * c + 20, :]
            w2a[j, 32 * c:32 * c + 20, 77 + c] = hw3[4 * j + c, :, 0]  # s3 head
    p["w2a"] = w2a

    # s1: term j covers h1 rows 320j..320j+319 across 3 tiles; strips hold
    # 3 full-M 128-col blocks (tile-aligned, zero-padded)
    S1_BASE_T = [0, 2, 5, 7]

    def s1_cols(j, o):
        R = 320 * j + o
        return 128 * (R // 128 - S1_BASE_T[j]) + R % 128

    cols308 = {j: np.array([s1_cols(j, o) for o in range(308)]) for j in range(4)}
    w1g = np.zeros((32, 128, 384), np.float32)
    for j in range(4):
        for c in range(8):
            w1g[8 * j + c][:, cols308[j]] = W1[j, 308 + 128 * c:308 + 128 * c + 128, :]
    p["w1g"] = w1g

    w1a = np.zeros((16, 128, 384), np.float32)
    for j in range(4):
        for c in range(4):
            w1a[4 * j + c][0:77, cols308[j]] = W1[j, 77 * c:77 * c + 77, :]
            w1a[4 * j + c][0:77, s1_cols(j, 308 + c)] = hw2[4 * j + c, :, 0]
    p["w1a"] = w1a

    # s0 gene: split into two 5-out-tile sweep tensors (each streamed once)
    w0g = np.zeros((32, 128, 1280), np.float32)
    for c in range(32):
        w0g[c, :, 0:1229] = W0[0, 1232 + 128 * c:1232 + 128 * c + 128, :]
    p["w0ga"] = np.ascontiguousarray(w0g[:, :, :640])
    p["w0gb"] = np.ascontiguousarray(w0g[:, :, 640:])

    # s0 act: [10, 128, 1280]; rows = act1T (4 terms x 320, 308 real)
    w0a = np.zeros((10, 128, 1280), np.float32)
    for c in range(10):
        for r in range(128):
            R = 128 * c + r
            j, rr = R // 320, R % 320
            if rr < 308:
                w0a[c, r, 0:1229] = W0[0, 308 * j + rr, :]
                w0a[c, r, 1229 + j] = hw1[j, rr, 0]  # s1 head
    p["w0aa"] = np.ascontiguousarray(w0a[:, :, :640])
    p["w0ab"] = np.ascontiguousarray(w0a[:, :, 640:])

    # head-extraction selection matrices
    sel3 = np.zeros((16, 128, 128), np.float32)
    for j2 in range(16):
        for jj in range(4):
            for c in range(4):
                sel3[j2, 32 * jj + 20 + c, 16 * (j2 % 8) + 4 * jj + c] = 1.0
    p["sel3"] = sel3
    sel2 = np.zeros((16, 128, 128), np.float32)
    for j in range(16):
        for c in range(4):
            sel2[j, 77 + c, 4 * j + c] = 1.0
    p["sel2"] = sel2
    sel1 = np.zeros((4, 128, 128), np.float32)
    rowk = [52, 116, 52, 116]
    for k in range(4):
        for c in range(4):
            sel1[k, rowk[k] + c, 64 + 4 * k + c] = 1.0
    p["sel1"] = sel1
    sel0 = np.zeros((1, 128, 128), np.float32)
    for c in range(4):
        sel0[0, 77 + c, 80 + c] = 1.0
    p["sel0"] = sel0

    hw0p = np.zeros((10, 128, 1), np.float32)
    for c in range(10):
        n = min(128, 1229 - 128 * c)
        if n > 0:
            hw0p[c, :n, 0] = hw0[0, 128 * c:128 * c + n, 0]
    p["hw0p"] = hw0p

    p["eye"] = np.eye(128, dtype=np.float32)[None]

    out16 = {}
    for k, v in p.items():
        v16 = v.astype(np.float16)
        if k in _ILV_N:
            v16 = _ilv(v16, _ILV_N[k])
        out16[k] = np.ascontiguousarray(v16)

    # g / bb channel-tiled f32 vectors [128, ntiles] (pad rows -> 0)
    def tile_vec(vec_f, s):
        nt = NTILES[s]
        slot = SLOT[s]
        out = np.zeros((128, nt), np.float32)
        for T in range(nt):
            for prt in range(128):
                R = 128 * T + prt
                j, o = R // slot, R % slot
                v = vec_f(j, o)
                if v is not None:
                    out[prt, T] = v
        return out

    gts, bbs = [], []
    for s, T_, I_, O_, _ in CFG:
        g, bb = inp[f"g{s}"], inp[f"bb{s}"]
        gts.append(tile_vec(lambda j, o: g[j, o] if (j < T_ and o < O_) else None, s))
        bbs.append(tile_vec(lambda j, o: bb[j, o] if (j < T_ and o < O_) else None, s))
    out16["gtall"] = np.ascontiguousarray(np.concatenate(gts, axis=1))
    out16["bball"] = np.ascontiguousarray(np.concatenate(bbs, axis=1))
    return out16


def _ilv(arr, n):
    """[nb, 128, F] -> [nb//n, 128, n*F]: n strips side-by-side per partition
    (matches the SBUF tile layout, so DMAs need no rearrange and get one
    contiguous chunk per partition)."""
    nb, p, f = arr.shape
    assert nb % n == 0
    return np.ascontiguousarray(
        arr.reshape(nb // n, n, p, f).transpose(0, 2, 1, 3).reshape(nb // n, p, n * f))

_ILV_N = {"w4p": 4, "w3g": 4, "w3a": 8, "w2g": 4, "w2a": 4, "w1g": 8,
          "w1a": 4, "w0ga": 2, "w0gb": 2, "w0aa": 2, "w0ab": 2, "sel3": 4, "sel2": 4,
          "sel1": 4, "hw0p": 10}


# ============================================================
# Bass program (built once, shared across calls)
# ============================================================
_NC = None

# s1 out-piece map: term j -> list of (tile, row_base, width) covering rows 320j..320j+319
S1_PIECES = {
    0: [(0, 0, 128), (1, 0, 128), (2, 0, 64)],
    1: [(2, 64, 64), (3, 0, 128), (4, 0, 128)],
    2: [(5, 0, 128), (6, 0, 128), (7, 0, 64)],
    3: [(7, 64, 64), (8, 0, 128), (9, 0, 128)],
}
TILE_LAST_TERM = {0: 0, 1: 0, 2: 1, 3: 1, 4: 1, 5: 2, 6: 2, 7: 3, 8: 3, 9: 3}
TILE_FIRST_TERM = {0: 0, 1: 0, 2: 0, 3: 1, 4: 1, 5: 2, 6: 2, 7: 2, 8: 3, 9: 3}
S1_BASE_T = [0, 2, 5, 7]


def _build():
    nc = bacc.Bacc("TRN2", target_bir_lowering=False, debug=False,
                   enable_asserts=True, num_devices=N_CORES)
    dep = bass._add_dep_helper
    io = {}
    io["xt"] = nc.dram_tensor("xt", [4, 128, 8 * BS], F16, kind="ExternalInput")
    for name, shp in [("w4p", [8, 128, 1024]), ("w3g", [8, 128, 512]),
                      ("w3a", [8, 128, 1024]), ("w2g", [8, 128, 512]),
                      ("w2a", [4, 128, 512]), ("w1g", [4, 128, 3072]),
                      ("w1a", [4, 128, 1536]), ("w0ga", [16, 128, 1280]),
                      ("w0gb", [16, 128, 1280]), ("w0aa", [5, 128, 1280]),
                      ("w0ab", [5, 128, 1280]),
                      ("sel3", [4, 128, 512]), ("sel2", [4, 128, 512]),
                      ("sel1", [1, 128, 512]), ("sel0", [1, 128, 128]),
                      ("hw0p", [1, 128, 10]), ("eye", [1, 128, 128])]:
        io[name] = nc.dram_tensor(name, shp, F16, kind="ExternalInput")
    NTOT = sum(NTILES.values())
    io["gtall"] = nc.dram_tensor("gtall", [128, NTOT], F32, kind="ExternalInput")
    io["bball"] = nc.dram_tensor("bball", [128, NTOT], F32, kind="ExternalInput")
    y = nc.dram_tensor("y", [4, 128, BS], F32, kind="ExternalOutput")

    rg = [list(range(N_CORES))]

    with tile.TileContext(nc, num_cores=N_CORES) as tc:
        with tc.tile_pool(name="per", bufs=1) as per, \
             tc.tile_pool(name="wp", bufs=3) as wp, \
             tc.tile_pool(name="pp", bufs=7, space="PSUM") as pp, \
             tc.tile_pool(name="dp", bufs=1, space="DRAM") as dp:

            # ---- persistent SBUF ----
            xsb = per.tile([128, 32 * BS], F16, name="xsb", tag="xsb")
            actT = {s: per.tile([128, NTILES[s] * BS], F16, name=f"act{s}", tag=f"act{s}")
                    for s in (4, 3, 2, 1, 0)}
            hT = {s: per.tile([128, NTILES[s] * BS], F16, name=f"h{s}", tag=f"h{s}")
                  for s in (3, 2, 1, 0)}
            h1gene = per.tile([128, 10 * BS], F16, name="h1gene", tag="h1gene")
            h0gene = per.tile([128, 10 * BS], F16, name="h0gene", tag="h0gene")
            stats = {s: per.tile([128, NTILES[s] * 6], F32, name=f"st{s}", tag=f"st{s}")
                     for s in (4, 3, 2, 1, 0)}
            agg = per.tile([128, 8 * NTILES[4] * 2], F32, name="agg", tag="agg")
            ccs = per.tile([128, NTILES[4] * 2], F32, name="ccs", tag="ccs")
            prtmp = per.tile([128, NTILES[4] * 2], F32, name="prtmp", tag="prtmp")
            prtmp2 = per.tile([128, NTILES[4]], F32, name="prtmp2", tag="prtmp2")
            aT = {s: per.tile([128, NTILES[s]], F32, name=f"aT{s}", tag=f"aT{s}")
                  for s in (4, 3, 2, 1, 0)}
            cT = {s: per.tile([128, NTILES[s]], F32, name=f"cT{s}", tag=f"cT{s}")
                  for s in (4, 3, 2, 1, 0)}
            sd_t = {s: per.tile([128, NTILES[s]], F32, name=f"sd{s}", tag=f"sd{s}")
                    for s in (4, 3, 2, 1, 0)}
            mub = per.tile([128, NTILES[4]], F32, name="mub", tag="mub")
            varb = per.tile([128, NTILES[4]], F32, name="varb", tag="varb")
            gsb_all = per.tile([128, 116], F32, name="gsb_all", tag="gsb_all")
            bbsb_all = per.tile([128, 116], F32, name="bbsb_all", tag="bbsb_all")
            _off = {}
            _o = 0
            for s in (4, 3, 2, 1, 0):
                _off[s] = _o
                _o += NTILES[s]
            gsb = {s: gsb_all[:, _off[s]:_off[s] + NTILES[s]] for s in (4, 3, 2, 1, 0)}
            bbsb = {s: bbsb_all[:, _off[s]:_off[s] + NTILES[s]] for s in (4, 3, 2, 1, 0)}
            outsb = per.tile([128, 4, BS], F32, name="outsb", tag="outsb")
            zbuf = per.tile([128, 8 * BS], F16, name="zbuf", tag="zbuf")
            eps_sb = per.tile([128, 1], F32, name="eps_sb", tag="eps_sb")
            eyesb = per.tile([128, 128], F16, name="eyesb", tag="eyesb")
            nc.vector.memset(eps_sb[:], EPS)
            nc.vector.memset(outsb[:, 3, :], 0.0)

            # ---- front DMAs: xt spread over 3 queues (ACT kept for w4t) ----
            nc.sync.dma_start(xsb[:, 0:8 * BS], io["xt"][0])
            nc.gpsimd.dma_start(xsb[:, 8 * BS:16 * BS], io["xt"][1])
            nc.gpsimd.dma_start(xsb[:, 16 * BS:24 * BS], io["xt"][2])
            nc.gpsimd.dma_start(xsb[:, 24 * BS:32 * BS], io["xt"][3])

            def xtile(t):
                return xsb[:, BS * t:BS * (t + 1)]

            def acttile(s, t):
                return actT[s][:, BS * t:BS * (t + 1)]

            def htile(s, t):
                return hT[s][:, BS * t:BS * (t + 1)]

            def chain(mms):
                for a, b in zip(mms, mms[1:]):
                    dep(b.ins, a.ins, sync=False,
                        reason="psum accumulation order")

            # slim stats prereduce + collective + postmath
            # stats layout: one 6-field record per tile, halves of 128:
            # (c0, m0, M2_0, c1, m1, M2_1)
            def prereduce(s, lo=0, n=None, coff=0):
                n = NTILES[s] if n is None else n
                sv = stats[s][:, 6 * lo:6 * (lo + n)].rearrange(
                    "p (t e th) -> p t e th", e=2, th=3)
                mv = sv[:, :, :, 1]   # per-half means (128 samples) [128, n, 2]
                vv = sv[:, :, :, 2]   # per-half M2 sums             [128, n, 2]
                msq = prtmp[:, :2 * n].rearrange("p (t e) -> p t e", e=2)
                f0 = ccs[:, coff:coff + n]
                f1 = ccs[:, coff + n:coff + 2 * n]
                nc.vector.tensor_tensor(msq, mv, mv, op=mybir.AluOpType.mult)
                nc.vector.tensor_tensor(f0, mv[:, :, 0], mv[:, :, 1],
                                        op=mybir.AluOpType.add)
                nc.vector.tensor_tensor(prtmp2[:, :n], msq[:, :, 0], msq[:, :, 1],
                                        op=mybir.AluOpType.add)
                nc.vector.tensor_scalar_mul(prtmp2[:, :n], prtmp2[:, :n], 128.0)
                nc.vector.tensor_tensor(f1, vv[:, :, 0], vv[:, :, 1],
                                        op=mybir.AluOpType.add)
                nc.vector.tensor_tensor(f1, f1, prtmp2[:, :n],
                                        op=mybir.AluOpType.add)

            def bn_collective(s, n=None, coff=0, aoff=0, suf=""):
                n = NTILES[s] if n is None else n
                F = 2 * n
                cc_in = dp.tile([128, F], F32, name=f"cci{s}{suf}")
                cc_out = dp.tile([N_CORES, 128, F], F32, name=f"cco{s}{suf}",
                                 addr_space="Shared")
                nc.gpsimd.dma_start(cc_in[:], ccs[:, coff:coff + F])
                nc.gpsimd.collective_compute(
                    "AllGather", mybir.AluOpType.bypass, replica_groups=rg,
                    ins=[cc_in.opt()], outs=[cc_out.opt()])
                nc.gpsimd.dma_start(agg[:, aoff:aoff + 8 * F],
                                    cc_out.rearrange("c p f -> p c f"))

            def postmath(s, lo=0, n=None, aoff=0):
                n = NTILES[s] if n is None else n
                F = 2 * n
                ag = agg[:, aoff:aoff + 8 * F]
                nc.vector.tensor_tensor(ag[:, 0:4 * F], ag[:, 0:4 * F],
                                        ag[:, 4 * F:8 * F], op=mybir.AluOpType.add)
                nc.vector.tensor_tensor(ag[:, 0:2 * F], ag[:, 0:2 * F],
                                        ag[:, 2 * F:4 * F], op=mybir.AluOpType.add)
                nc.vector.tensor_tensor(ag[:, 0:F], ag[:, 0:F],
                                        ag[:, F:2 * F], op=mybir.AluOpType.add)
                mu = mub[:, lo:lo + n]
                va = varb[:, lo:lo + n]
                sd = sd_t[s][:, lo:lo + n]
                nc.vector.tensor_scalar_mul(mu, ag[:, 0:n], 1.0 / 16)
                nc.vector.tensor_scalar_mul(va, ag[:, n:F], 1.0 / 2048)
                nc.vector.tensor_tensor(prtmp2[:, :n], mu, mu,
                                        op=mybir.AluOpType.mult)
                nc.vector.tensor_tensor(va, va, prtmp2[:, :n],
                                        op=mybir.AluOpType.subtract)
                nc.scalar.activation(sd, va,
                                     mybir.ActivationFunctionType.Sqrt,
                                     bias=eps_sb[:, 0:1])
                nc.vector.reciprocal(sd, sd)
                nc.vector.tensor_tensor(aT[s][:, lo:lo + n], sd,
                                        gsb[s][:, lo:lo + n],
                                        op=mybir.AluOpType.mult)
                nc.vector.tensor_tensor(sd, mu, aT[s][:, lo:lo + n],
                                        op=mybir.AluOpType.mult)
                nc.vector.tensor_tensor(cT[s][:, lo:lo + n],
                                        bbsb[s][:, lo:lo + n], sd,
                                        op=mybir.AluOpType.subtract)

            def tanh_tile(s, t, src):
                nc.scalar.activation(
                    acttile(s, t), src,
                    mybir.ActivationFunctionType.Tanh,
                    bias=cT[s][:, t:t + 1], scale=aT[s][:, t:t + 1])

            def copy_stat(s, t, ps):
                # h tile: PSUM f32 -> SBUF fp16 (gpsimd cannot read PSUM),
                # per-tile fp16 bn_stats on DVE. For s3 the ACT queue is the
                # tanh-critical pacer in gap4, so its copies all go to DVE;
                # other strata alternate ACT/DVE.
                if s != 3 and t % 2 == 0:
                    nc.scalar.copy(htile(s, t), ps[:])
                else:
                    nc.vector.tensor_copy(htile(s, t), ps[:])
                nc.vector.bn_stats(stats[s][:, 6 * t:6 * t + 6], htile(s, t))

            # ================= s4 pass 1: stats only =================
            def s4_mm(t, ps, wt):
                a = t % 2
                rhs = xsb[64 * a:64 * a + 64, BS * (t // 2):BS * (t // 2) + BS]
                return nc.tensor.matmul(
                    ps[:], wt[64 * a:64 * a + 64, 128 * a:128 * a + 128],
                    rhs, start=True, stop=True)

            w4keep = {}

            def s4p1_range(P0, P1):
                for P in range(P0, P1):
                    if P % 4 == 0:
                        w4t = wp.tile([128, 1024], F16, name="w4t", tag="w4",
                                      bufs=8)
                        nc.sync.dma_start(w4t[:], io["w4p"][P // 4])
                        w4keep[P // 4] = w4t
                    w4v = w4t[:, 256 * (P % 4):256 * (P % 4) + 256]
                    for a in range(2):
                        t = 2 * P + a
                        ps = pp.tile([128, BS], F32, name="ps4", tag="ps")
                        s4_mm(t, ps, w4v)
                        if t % 4 == 0:
                            # direct f32 stats from PSUM on DVE (no copy)
                            nc.vector.bn_stats(stats[4][:, 6 * t:6 * t + 6],
                                               ps[:])
                        else:
                            sc = acttile(4, t)
                            nc.scalar.copy(sc, ps[:])
                            nc.vector.bn_stats(stats[4][:, 6 * t:6 * t + 6], sc)

            # s4 stats in two halves; half-A collective fires at mid-front so
            # half-A tanh + s3-A overlap the half-B collective
            s4p1_range(0, 16)
            nc.sync.dma_start(gsb_all[:], io["gtall"][:])
            nc.sync.dma_start(bbsb_all[:], io["bball"][:])
            nc.sync.dma_start(eyesb[:], io["eye"][0])
            prereduce(4, lo=0, n=32, coff=0)
            bn_collective(4, n=32, coff=0, aoff=0, suf="a")
            s4p1_range(16, 32)
            prereduce(4, lo=32, n=32, coff=64)
            bn_collective(4, n=32, coff=64, aoff=512, suf="b")

            # ================= s1 gene (under coll4) =================
            ps1 = {}
            mms1 = {}
            for j in range(4):
                for (tl, rb, w) in S1_PIECES[j]:
                    if tl not in ps1:
                        ps1[tl] = pp.tile([128, BS], F32, name=f"ps1g{tl}", tag="ps")
                        mms1[tl] = []
                w1t = wp.tile([128, 8 * 384], F16, name="w1t", tag="w1", bufs=2)
                nc.sync.dma_start(w1t[:], io["w1g"][j])
                for c in range(8):
                    for (tl, rb, w) in S1_PIECES[j]:
                        lt = tl - S1_BASE_T[j]
                        mms1[tl].append(nc.tensor.matmul(
                            ps1[tl][:], w1t[:, 384 * c + 128 * lt:384 * c + 128 * lt + 128],
                            xtile(8 * j + c),
                            start=(c == 0 and j == TILE_FIRST_TERM[tl]),
                            stop=(c == 7 and j == TILE_LAST_TERM[tl])))
                for tl, lt in TILE_LAST_TERM.items():
                    if lt == j and tl in ps1:
                        chain(mms1[tl])
                        if tl % 2 == 0:
                            nc.scalar.copy(h1gene[:, BS * tl:BS * (tl + 1)],
                                           ps1[tl][:])
                        else:
                            nc.vector.tensor_copy(
                                h1gene[:, BS * tl:BS * (tl + 1)], ps1[tl][:])
                        del ps1[tl]


            # ================= gap4: s4 pass 2 + s3 interleaved =================
            # (w4p pass 2 / w3g / w3a stream on the SP queue, idle in this phase)
            def gap4_range(P0, P1):
              for P in range(P0, P1):
                w4t2 = w4keep[P // 4]
                w4v2 = w4t2[:, 256 * (P % 4):256 * (P % 4) + 256]
                for a in range(2):
                    t = 2 * P + a
                    ps = pp.tile([128, BS], F32, name="ps4b", tag="ps")
                    s4_mm(t, ps, w4v2)
                    if P % 2 == 0:
                        # first half of each 4-tile group: DVE prescale into
                        # zbuf (fast PSUM release), wide plain tanh later
                        zs = (t // 4 % 2) * 2 + t % 4
                        zslot = zbuf[:, BS * zs:BS * (zs + 1)]
                        nc.vector.tensor_scalar(zslot, ps[:], aT[4][:, t:t + 1],
                                                cT[4][:, t:t + 1],
                                                op0=mybir.AluOpType.mult,
                                                op1=mybir.AluOpType.add)
                    else:
                        # second half: fused scale/bias tanh from PSUM on ACT
                        tanh_tile(4, t, ps[:])
                if P % 2 == 1:
                    t3 = P // 2
                    # plain wide tanh over the 2 prescaled tiles 4*t3, 4*t3+1
                    h = ((t3 % 2) * 2) * BS
                    nc.scalar.activation(
                        actT[4][:, BS * 4 * t3:BS * (4 * t3 + 2)],
                        zbuf[:, h:h + 2 * BS],
                        mybir.ActivationFunctionType.Tanh)
                    if t3 % 2 == 0:
                        w3at = wp.tile([128, 1024], F16, name="w3at", tag="w3a", bufs=4)
                        nc.sync.dma_start(w3at[:], io["w3a"][t3 // 2])
                        w3gt = wp.tile([128, 512], F16, name="w3gt", tag="w3g", bufs=4)
                        nc.sync.dma_start(w3gt[:], io["w3g"][t3 // 2])
                    ps = pp.tile([128, BS], F32, name="ps3", tag="ps")
                    mms = []
                    for a in range(2):  # gene groups (full-M padded)
                        G = 2 * t3 + a
                        goff = 128 * (G % 4)
                        mms.append(nc.tensor.matmul(
                            ps[:], w3gt[:, goff:goff + 128], xtile(G),
                            start=(a == 0), stop=False))
                    for jj in range(4):  # act terms (full-M padded)
                        j = 4 * t3 + jj
                        k = 128 * (j % 8)
                        mms.append(nc.tensor.matmul(
                            ps[:], w3at[:, k:k + 128],
                            acttile(4, j), start=False, stop=(jj == 3)))
                    chain(mms)
                    copy_stat(3, t3, ps)
            postmath(4, lo=0, n=32, aoff=0)
            gap4_range(0, 16)
            postmath(4, lo=32, n=32, aoff=512)
            gap4_range(16, 32)

            prereduce(3)
            bn_collective(3)

            # ================= under coll3: s4 heads + s0g sweep A =================
            w2ah = wp.tile([128, 4 * 512], F16, name="w2ah", tag="w2a", bufs=1)
            st3h = wp.tile([128, 4 * 512], F16, name="st3h", tag="sel3", bufs=1)
            for n in range(4):
                nc.scalar.dma_start(w2ah[:, 512 * n:512 * (n + 1)], io["w2a"][n])
                nc.scalar.dma_start(st3h[:, 512 * n:512 * (n + 1)], io["sel3"][n])

            psA = pp.tile([128, BS], F32, name="psA", tag="ps")
            psB = pp.tile([128, BS], F32, name="psB", tag="ps")
            mmsA, mmsB = [], []
            for j2 in range(16):
                mm = nc.tensor.matmul(
                    (psA if j2 < 8 else psB)[:],
                    st3h[:, 512 * (j2 // 4) + 128 * (j2 % 4):
                         512 * (j2 // 4) + 128 * (j2 % 4) + 128],
                    htile(3, j2),
                    start=(j2 % 8 == 0), stop=(j2 % 8 == 7))
                (mmsA if j2 < 8 else mmsB).append(mm)
            chain(mmsA)
            chain(mmsB)
            nc.scalar.copy(outsb[:, 0, :], psA[:])
            nc.scalar.copy(outsb[:, 1, :], psB[:])
            nc.scalar.dma_start(y[0], outsb[:, 0, :])
            nc.scalar.dma_start(y[1], outsb[:, 1, :])

            # s0g sweep A: fills the coll3 window
            psga = [pp.tile([128, BS], F32, name=f"ps0ga{m}", tag="ps")
                    for m in range(5)]
            mmsga = [[] for _ in range(5)]
            for c in range(32):
                if c % 2 == 0:
                    w0t = wp.tile([128, 1280], F16, name="w0t", tag="w0", bufs=3)
                    nc.sync.dma_start(w0t[:], io["w0ga"][c // 2])
                base = 640 * (c % 2)
                for m in range(5):
                    mmsga[m].append(nc.tensor.matmul(
                        psga[m][:], w0t[:, base + 128 * m:base + 128 * m + 128],
                        xtile(c), start=(c == 0), stop=(c == 31)))
            for m in range(5):
                chain(mmsga[m])
                nc.scalar.copy(h0gene[:, BS * m:BS * (m + 1)], psga[m][:])

            postmath(3)

            # ================= gap3: act3 + s2 =================
            for j in range(16):
                if j % 2 == 0:
                    w2gt = wp.tile([128, 512], F16, name="w2gt", tag="w2g", bufs=2)
                    nc.gpsimd.dma_start(w2gt[:], io["w2g"][j // 2])
                tanh_tile(3, j, htile(3, j))
                ps = pp.tile([128, BS], F32, name="ps2", tag="ps")
                mms = []
                for c2 in range(2):
                    goff = 128 * ((2 * j + c2) % 4)
                    mms.append(nc.tensor.matmul(
                        ps[:], w2gt[:, goff:goff + 128], xtile(2 * j + c2),
                        start=(c2 == 0), stop=False))
                aoff = 512 * (j // 4) + 128 * (j % 4)
                mms.append(nc.tensor.matmul(
                    ps[:], w2ah[:, aoff:aoff + 128], acttile(3, j),
                    start=False, stop=True))
                chain(mms)
                copy_stat(2, j, ps)
            # s0g sweep B: fills the cci2/coll2 window
            psgb = [pp.tile([128, BS], F32, name=f"ps0gb{m}", tag="ps")
                    for m in range(5)]
            mmsgb = [[] for _ in range(5)]
            for c in range(32):
                if c % 2 == 0:
                    w0t = wp.tile([128, 1280], F16, name="w0tb", tag="w0", bufs=3)
                    nc.sync.dma_start(w0t[:], io["w0gb"][c // 2])
                base = 640 * (c % 2)
                for m in range(5):
                    mmsgb[m].append(nc.tensor.matmul(
                        psgb[m][:], w0t[:, base + 128 * m:base + 128 * m + 128],
                        xtile(c), start=(c == 0), stop=(c == 31)))

            prereduce(2)
            bn_collective(2)

            # ================= under coll2: s0g sweep B + s3 heads =================
            st2h = wp.tile([128, 4 * 512], F16, name="st2h", tag="sel2", bufs=1)
            w1ah = wp.tile([128, 4 * 1536], F16, name="w1ah", tag="w1a", bufs=1)
            for n in range(4):
                nc.scalar.dma_start(st2h[:, 512 * n:512 * (n + 1)], io["sel2"][n])
                nc.sync.dma_start(w1ah[:, 1536 * n:1536 * (n + 1)], io["w1a"][n])

            psC = pp.tile([128, BS], F32, name="psC", tag="psC", bufs=1)
            mmsC = []
            for j in range(16):  # s3 heads from h2
                mmsC.append(nc.tensor.matmul(
                    psC[:], st2h[:, 512 * (j // 4) + 128 * (j % 4):
                                 512 * (j // 4) + 128 * (j % 4) + 128],
                    htile(2, j),
                    start=(j == 0), stop=False))
            for m in range(5):
                chain(mmsgb[m])
                nc.scalar.copy(h0gene[:, BS * (5 + m):BS * (6 + m)], psgb[m][:])

            postmath(2)

            # ================= gap2: act2 + s1 act =================
            for j in range(16):
                tanh_tile(2, j, htile(2, j))
            ps1a = {}
            mms1a = {}
            for j in range(4):
                for (tl, rb, w) in S1_PIECES[j]:
                    if tl not in ps1a:
                        ps1a[tl] = pp.tile([128, BS], F32, name=f"ps1a{tl}", tag="ps")
                        mms1a[tl] = []
                for c in range(4):
                    for (tl, rb, w) in S1_PIECES[j]:
                        lt = tl - S1_BASE_T[j]
                        mms1a[tl].append(nc.tensor.matmul(
                            ps1a[tl][:],
                            w1ah[:, 1536 * j + 384 * c + 128 * lt:
                                 1536 * j + 384 * c + 128 * lt + 128],
                            acttile(2, 4 * j + c),
                            start=(c == 0 and j == TILE_FIRST_TERM[tl]),
                            stop=False))
                for tl, lt in TILE_LAST_TERM.items():
                    if lt == j and tl in ps1a:
                        mms1a[tl].append(nc.tensor.matmul(
                            ps1a[tl][:], eyesb[:],
                            h1gene[:, BS * tl:BS * (tl + 1)],
                            start=False, stop=True))
                        chain(mms1a[tl])
                        copy_stat(1, tl, ps1a[tl])
                        del ps1a[tl]
            prereduce(1)
            bn_collective(1)

            # ================= under coll1: s2 heads + w0a load =================
            st1 = wp.tile([128, 512], F16, name="st1", tag="sel", bufs=1)
            nc.scalar.dma_start(st1[:], io["sel1"][0])
            st0 = wp.tile([128, 128], F16, name="st0", tag="sel0", bufs=1)
            nc.scalar.dma_start(st0[:], io["sel0"][0])
            hw0t = wp.tile([128, 10], F16, name="hw0t", tag="hw0", bufs=1)
            nc.scalar.dma_start(hw0t[:], io["hw0p"][0])
            for k, tl in enumerate((2, 4, 7, 9)):  # s2 heads from h1
                mmsC.append(nc.tensor.matmul(
                    psC[:], st1[:, 128 * k:128 * k + 128],
                    htile(1, tl),
                    start=False, stop=False))
            postmath(1)

            # ================= gap1: act1 + s0 act (one 10-PSUM pass) =================
            for wave, wname in ((0, "w0aa"), (1, "w0ab")):
                ps0 = [pp.tile([128, BS], F32, name=f"ps0a{wave}{i}", tag="ps")
                       for i in range(5)]
                mms0 = [[] for _ in range(5)]
                for k in range(10):
                    if k % 2 == 0:
                        w0at = wp.tile([128, 1280], F16, name="w0at", tag="w0",
                                       bufs=3)
                        nc.sync.dma_start(w0at[:], io[wname][k // 2])
                    if wave == 0:
                        tanh_tile(1, k, htile(1, k))
                    base = 640 * (k % 2)
                    for i in range(5):
                        mms0[i].append(nc.tensor.matmul(
                            ps0[i][:], w0at[:, base + 128 * i:base + 128 * i + 128],
                            acttile(1, k), start=(k == 0), stop=False))
                for i in range(5):
                    m = 5 * wave + i
                    mms0[i].append(nc.tensor.matmul(
                        ps0[i][:], eyesb[:], h0gene[:, BS * m:BS * (m + 1)],
                        start=False, stop=True))
                    chain(mms0[i])
                    copy_stat(0, m, ps0[i])
            # s1 heads from h0 tile 9
            mmsC.append(nc.tensor.matmul(
                psC[:], st0[:, :], htile(0, 9),
                start=False, stop=True))
            chain(mmsC)
            nc.scalar.copy(outsb[:, 2, :], psC[:])
            prereduce(0)
            bn_collective(0)
            nc.scalar.dma_start(y[2], outsb[:, 2, :])
            postmath(0)

            # ================= tail: act0 + s0 head =================
            psD = pp.tile([128, BS], F32, name="psD", tag="ps")
            mmsD = []
            for c in range(10):
                tanh_tile(0, c, htile(0, c))
                mmsD.append(nc.tensor.matmul(
                    psD[0:1, :], hw0t[:, c:c + 1], acttile(0, c),
                    start=(c == 0), stop=(c == 9)))
            chain(mmsD)
            nc.vector.tensor_copy(outsb[0:1, 3, :], psD[0:1, :])
            nc.scalar.dma_start(y[3], outsb[:, 3, :])

    nc.finalize()
    return nc


def kernel(**inputs):
    global _NC, LAST_RESULTS
    inputs = {k: np.asarray(v) for k, v in inputs.items()}
    packed = _pack(inputs)

    x = inputs["x"].astype(np.float32)
    if _NC is None:
        _NC = _build()

    in_maps = []
    for c in range(N_CORES):
        m = dict(packed)
        xs = x[BS * c:BS * (c + 1), :]                    # [256, 4096]
        xT = np.ascontiguousarray(xs.T.astype(np.float16))  # [4096, 256]
        m["xt"] = _ilv(xT.reshape(32, 128, BS).astype(np.float16), 8)
        in_maps.append(m)

    res = run_bass_kernel_spmd(_NC, in_maps, core_ids=list(range(N_CORES)))
    LAST_RESULTS = res

    hb_row = np.concatenate([inputs["hb4"][:, 0], inputs["hb3"][:, 0],
                             inputs["hb2"][:, 0], inputs["hb1"][:, 0],
                             inputs["hb0"][:, 0]]).astype(np.float32)  # [341]
    out = np.empty((B, 341), np.float32)
    for c in range(N_CORES):
        arr = res.results[c]["y"]  # [4, 128, 256]
        headsT = np.concatenate([arr[0], arr[1], arr[2][:84], arr[3][:1]], 0)  # [341, 256]
        out[BS * c:BS * (c + 1), :] = headsT.T + hb_row[None, :]
    return out
